# revision 11
# baseline (speedup 1.0000x reference)
"""NonLocalBlock (embedded-gaussian attention) TRN2 kernel, v3.

Shapes (hardcoded): x [8, 256, 64, 64] fp32, one batch element per core.
Per core:
  theta' = A * (theta_w^T x + theta_b)   (A = 128*log2(e); f' = A*f)
  phi/g  = 1x1 conv projections, [128, 4096]
  f'^T[j, i] = sum_c phi[c, j] theta'[c, i]   (A-scaled 4096x4096 logits)
  Scalar tiles: ef = Exp(f' * (1/A) + (-10))   (global bias -10)
  DVE tiles (9 of 128): Schraudolph bf16 fast-exp in ONE tensor_scalar:
      bits_i16 = trunc(max(f' + CB, 0)), viewed as bf16 ~= e^(f-10)
      (CB = 16251 - A*10; piecewise-linear 2^frac, +-3.3% per elem,
       validated 2.3e-3 rel err end-to-end vs 2e-2 gate)
  Z[i] = sum_j ef[j, i]  via bf16 DVE adds (binary tree then running),
         closed by ones-matmul partition reduce; 1/Z via fast reciprocal;
         Z broadcast via PE ones-row matmul into PSUM (not gpsimd: its
         tensor ops share the DVE SBUF port and inflate DVE ops ~2x)
  y[ci, i] = sum_j ef[j, i] gT[j, ci]   (bf16 matmuls, fp32 PSUM)
  out = x + W_w @ (y * (1/Z)) + wbe,  wbe = W_w @ g_b + W_b folded into
        the W-proj eviction via scalar_tensor_tensor (kills the 16
        bias-add DVE ops of v2)

Engine budget per t (cadence target ~1.03us): Scalar 119 exps x 1.08;
DVE Z-add 0.67 + evictions/mult/STT + 9 fast-exps x 1.24; PE f 0.43 +
y 0.43 + proj/wproj/zclose/bcast ~0.12.

Ramp: 3 DMA queues (sync/scalar hw + gpsimd swdge for weights), first x
pieces 256-col, th0 in 2x256 pieces, t=0/1 exps split in halves so the
Scalar pipe starts ~13.2us instead of 21us.

Tail: y-lag ramps 11->5 over Q3 (extra y slots), then half-pipelined
zclose->zinv->bcastPE->mult->wproj chains alternating DMA queues.
"""

import numpy as np

import concourse.bacc as bacc
import concourse.mybir as mybir
from concourse import tile
from concourse.bass_utils import run_bass_kernel_spmd

F32 = mybir.dt.float32
F32R = mybir.dt.float32r
BF16 = mybir.dt.bfloat16
I16 = mybir.dt.int16
AF = mybir.ActivationFunctionType
OP = mybir.AluOpType

B, C, CI = 8, 256, 128
H, Wd = 64, 64
N = H * Wd              # 4096
NQ = 4                  # i-quarters
QW = N // NQ            # 1024
JB = N // 128           # 32 j-blocks (= ts per quarter)
T = NQ * JB             # 128

YLAG = 11               # steady-state y-matmul lag behind f/exp
YEXTRA = {17, 19, 21, 23, 25, 27}   # q3 slots emitting a 2nd y (lag 11->5)
ZCLOSE_J = (3, 4)       # prev quarter's Z partition-reduce MMs
ZINV_J = 5              # prev quarter's reciprocals
BCAST_J = (9, 10)       # prev quarter's Z broadcast (PE ones-row MM)
MULT_J = (11, 12)       # prev quarter's normalize halves
WPROJ_J0 = 13           # prev quarter's W-projection chunks j=13..16
ZRUN_J = 24             # switch Z accumulation from tree to running adds

EXP_BIAS = -10.0        # global logit shift (f range [-90.8, 84.8])
A_SCALE = 128.0 * 1.4426950408889634   # 184.66496...
CB = np.float32(16251.0 + A_SCALE * EXP_BIAS)  # schraudolph add const
SCL = np.float32(1.0 / A_SCALE)
NWARM = 4

# (q, j) tiles whose exp runs on DVE (fast-exp); chosen away from
# deferred-op j slots and sched-heavy ts.  k=9.
DVE_SET = {(1, 20), (1, 28),
           (2, 8), (2, 20), (2, 28),
           (3, 2), (3, 8), (3, 20), (3, 26)}


def _build_sched():
    # t -> list of ops. pieces are 512-col units c=0..7 of x columns.
    # deadlines: ph piece c before f uses j-block 4c (t=4c); th piece c
    # before f of quarter c//2 (t=32*(c//2)); g piece c before y uses
    # block 4c (t=YLAG+4c).  j in {9..16} of q>=1 avoided (pw pool is
    # busy with zb/wproj there).
    sched = {}

    def add(t, op):
        sched.setdefault(t, []).append(op)

    for c in range(1, 8):
        add(4 * c - 3, ("ph", c))
    for c, t in ((2, 24), (3, 26), (4, 50), (5, 54), (6, 82), (7, 86)):
        add(t, ("th", c))
    gsched = {0: 2, 1: 6, 2: 10, 3: 14, 4: 18, 5: 22, 6: 30, 7: 33}
    for c, t in gsched.items():
        add(t, ("ga", c))
        add(t + 1, ("gb", c))
    return sched


def build():
    nc = bacc.Bacc("TRN2", target_bir_lowering=False, debug=False, num_devices=8)

    x_d = nc.dram_tensor("x", [C, N], F32R, kind="ExternalInput")
    thw_d = nc.dram_tensor("thw_t", [C, CI], F32R, kind="ExternalInput")  # theta_w.T
    phw_d = nc.dram_tensor("phw_t", [C, CI], F32R, kind="ExternalInput")  # phi_w.T
    gw_d = nc.dram_tensor("gw_t", [C, CI], F32R, kind="ExternalInput")    # g_w.T
    ww_d = nc.dram_tensor("ww_t", [CI, C], F32R, kind="ExternalInput")    # W_w.T
    # aux cols: 0=A*theta_b, 1=phi_b, 2=wbe[:128], 3=wbe[128:], 4=ones,
    # 5=exp bias (-10)
    aux_d = nc.dram_tensor("aux", [128, 6], F32, kind="ExternalInput")
    out_d = nc.dram_tensor("out", [C, N], F32, kind="ExternalOutput")

    sched = _build_sched()

    with tile.TileContext(nc) as tc:
        with (
            tc.tile_pool(name="const", bufs=1) as cpool,
            tc.tile_pool(name="big", bufs=1) as bigpool,
            tc.tile_pool(name="ef", bufs=13) as efpool,
            tc.tile_pool(name="ztree", bufs=2) as ztpool,
            tc.tile_pool(name="zpool", bufs=2) as zpool,
            tc.tile_pool(name="ypool", bufs=2) as ypool,
            tc.tile_pool(name="opool", bufs=6) as opool,
            tc.tile_pool(name="pf", bufs=2, space="PSUM") as pf,
            tc.tile_pool(name="py", bufs=1, space="PSUM") as py,
            tc.tile_pool(name="pw", bufs=2, space="PSUM") as pw,
        ):
            # ---------------- warmup + DMA issue ----------------
            warm = cpool.tile([128, 512], BF16, tag="warm")
            warm2 = cpool.tile([128, 1], F32, tag="warm2")
            nc.gpsimd.memset(warm[:], 0.0)

            aux = cpool.tile([128, 6], F32, tag="aux")
            thw = cpool.tile([128, 2 * CI], F32R, tag="thw")
            phw = cpool.tile([128, 2 * CI], F32R, tag="phw")
            gw = cpool.tile([128, 2 * CI], F32R, tag="gw")
            ww = cpool.tile([CI, C], F32R, tag="ww")
            x0 = bigpool.tile([128, N], F32R, tag="x0")
            x1 = bigpool.tile([128, N], F32R, tag="x1")
            xs = (x0, x1)

            # x on the two hw queues (256-col first pieces: the rings
            # ramp slowly and the first piece gates the first proj);
            # all weights on the gpsimd software-DGE queue.
            nc.sync.dma_start(aux[:], aux_d[:])
            nc.sync.dma_start(x0[:, 0:256], x_d[0:128, 0:256])
            nc.scalar.dma_start(x1[:, 0:256], x_d[128:256, 0:256])
            nc.sync.dma_start(thw[:, 0:CI], thw_d[0:128, :])
            nc.scalar.dma_start(thw[:, CI:2 * CI], thw_d[128:256, :])
            # dummy activation pulls the ~1.3us exp table load early
            nc.scalar.activation(warm2[:], warm[:, 0:1], AF.Identity)
            nc.sync.dma_start(x0[:, 256:512], x_d[0:128, 256:512])
            nc.scalar.dma_start(x1[:, 256:512], x_d[128:256, 256:512])
            nc.sync.dma_start(phw[:, 0:CI], phw_d[0:128, :])
            nc.scalar.dma_start(phw[:, CI:2 * CI], phw_d[128:256, :])
            nc.sync.dma_start(x0[:, 512:1024], x_d[0:128, 512:1024])
            nc.scalar.dma_start(x1[:, 512:1024], x_d[128:256, 512:1024])
            nc.sync.dma_start(gw[:, 0:CI], gw_d[0:128, :])
            nc.scalar.dma_start(gw[:, CI:2 * CI], gw_d[128:256, :])
            nc.sync.dma_start(x0[:, 1024:2048], x_d[0:128, 1024:2048])
            nc.scalar.dma_start(x1[:, 1024:2048], x_d[128:256, 1024:2048])
            nc.sync.dma_start(ww[:], ww_d[:])
            nc.sync.dma_start(x0[:, 2048:4096], x_d[0:128, 2048:4096])
            nc.scalar.dma_start(x1[:, 2048:4096], x_d[128:256, 2048:4096])

            for _ in range(NWARM):
                pwt = pw.tile([128, 512], F32, tag="pw", name="warm_mm")
                nc.tensor.matmul(pwt[:], warm[:, 0:128], warm[:],
                                 start=True, stop=True)

            thbA = aux[:, 0:1]   # A * theta_b
            phb = aux[:, 1:2]
            wbe = (aux[:, 2:3], aux[:, 3:4])
            ones_bf = cpool.tile([128, 1], BF16, tag="ones_bf")
            nc.vector.tensor_copy(ones_bf[:], aux[:, 4:5])

            th_sb = bigpool.tile([128, N], F32R, tag="th")
            ph_sb = bigpool.tile([128, N], F32R, tag="ph")
            gT_sb = bigpool.tile([128, N], BF16, tag="gT")

            # ---------------- projection piece emitters ----------------
            def proj_mm(pp, wt, lo, w):
                for k in range(2):
                    nc.tensor.matmul(
                        pp[:, 0:w], wt[:, k * CI:(k + 1) * CI],
                        xs[k][:, lo:lo + w],
                        start=(k == 0), stop=(k == 1),
                    )

            def th_piece(lo, w, ev="v", pool_tag="pw"):
                # theta' = A*(proj + thb): evict applies scale A + A*thb
                pool = pf if pool_tag == "pf" else pw
                pp = pool.tile([128, 512], F32, tag=pool_tag,
                               name=f"thp_{lo}")
                proj_mm(pp, thw, lo, w)
                if ev == "s":
                    nc.scalar.activation(th_sb[:, lo:lo + w], pp[:, 0:w],
                                         AF.Identity, bias=thbA,
                                         scale=float(A_SCALE))
                else:
                    nc.vector.tensor_scalar(
                        th_sb[:, lo:lo + w], pp[:, 0:w],
                        float(A_SCALE), thbA, OP.mult, OP.add)

            def ph_piece(lo, w, ev="v", pool_tag="pw"):
                pool = pf if pool_tag == "pf" else pw
                pp = pool.tile([128, 512], F32, tag=pool_tag,
                               name=f"php_{lo}")
                proj_mm(pp, phw, lo, w)
                if ev == "s":
                    nc.scalar.activation(ph_sb[:, lo:lo + w], pp[:, 0:w],
                                         AF.Identity, bias=phb)
                else:
                    nc.vector.tensor_scalar_add(ph_sb[:, lo:lo + w],
                                                pp[:, 0:w], phb)

            gtiles = {}

            def g_piece(c, half):
                # gT blocks 4c+2*half, 4c+2*half+1 into shared [128,512] tile
                if half == 0:
                    gtiles[c] = pw.tile([128, 512], F32, tag="pw",
                                        name=f"gp_{c}")
                pg = gtiles[c]
                for b in (4 * c + 2 * half, 4 * c + 2 * half + 1):
                    col = (b - 4 * c) * 128
                    for k in range(2):
                        nc.tensor.matmul(
                            pg[:, col:col + 128],
                            xs[k][:, b * 128:(b + 1) * 128],
                            gw[:, k * CI:(k + 1) * CI],
                            start=(k == 0), stop=(k == 1),
                        )
                if half == 1:
                    lo = c * 512
                    nc.vector.tensor_copy(gT_sb[:, lo:lo + 512], pg[:])
                    del gtiles[c]

            def emit_sched_op(op):
                kind = op[0]
                if kind == "th":
                    th_piece(op[1] * 512, 512)
                elif kind == "ph":
                    ph_piece(op[1] * 512, 512)
                elif kind == "ga":
                    g_piece(op[1], 0)
                elif kind == "gb":
                    g_piece(op[1], 1)

            # ---------------- per-quarter deferred ops ----------------
            state = {}   # per-quarter: zq, pzt[2], zi, zbt[2], pyt, ynt
            efs = {}

            def zclose(q, s):
                st = state[q]
                pzt = pw.tile([1, 512], F32, tag="pw", name=f"pz_{q}_{s}")
                st["pzt"][s] = pzt
                nc.tensor.matmul(pzt[:], ones_bf[:],
                                 st["zq"][:, s * 512:(s + 1) * 512],
                                 start=True, stop=True)

            def zinv(q, s):
                # fast variant: ~18 correct bits; Z is far from the edges.
                # zi is F32R so the bcast matmul can consume it at 1 cyc/row
                # (the BIR verifier requires f32r matmul inputs to be
                # *written* as f32r); _custom_dve directly since the
                # wrapper asserts fp32 out.
                from concourse.dve_ops import (
                    RECIP_APPROX_FAST_CONSTS,
                    RECIPROCAL_APPROX_FAST,
                )
                st = state[q]
                if s == 0:
                    st["zi"] = zpool.tile([1, QW], F32R, tag="zi",
                                          name=f"zi_{q}")
                c = RECIP_APPROX_FAST_CONSTS
                nc.vector._custom_dve(
                    RECIPROCAL_APPROX_FAST,
                    out=st["zi"][:, s * 512:(s + 1) * 512],
                    in0=st["pzt"][s][:],
                    s0=c["s0"], s1=c["s1"], imm2=c["imm2"])

            def bcast_pe(q, s):
                # gpsimd partition broadcast (DVE tensor_tensor cannot read
                # two PSUM operands, so zb must live in SBUF; PE can only
                # write PSUM).  gpsimd is otherwise idle.
                st = state[q]
                if s == 0:
                    st["zb"] = zpool.tile([128, QW], F32, tag="zb",
                                          name=f"zb_{q}")
                nc.gpsimd.partition_broadcast(
                    st["zb"][:, s * 512:(s + 1) * 512],
                    st["zi"][:, s * 512:(s + 1) * 512].bitcast(F32))

            def mult(q, s):
                st = state[q]
                if s == 0:
                    st["ynt"] = ypool.tile([128, QW], F32R, tag="ynt",
                                           name=f"ynt_{q}")
                nc.vector.tensor_mul(
                    st["ynt"][:, s * 512:(s + 1) * 512],
                    st["pyt"][:, s * 512:(s + 1) * 512],
                    st["zb"][:, s * 512:(s + 1) * 512])

            def wproj(q, chunk, dma_eng=None):
                ob, s2 = divmod(chunk, 2)
                lo = q * QW + s2 * 512
                pwt = pw.tile([128, 512], F32, tag="pw",
                              name=f"pw_{q}_{chunk}")
                nc.tensor.matmul(
                    pwt[:], ww[:, ob * CI:(ob + 1) * CI],
                    state[q]["ynt"][:, s2 * 512:(s2 + 1) * 512],
                    start=True, stop=True)
                ot = opool.tile([128, 512], F32, tag="o", name=f"o_{q}_{chunk}")
                # out = (W@y/Z + wbe) + x  in one DVE op
                nc.vector.scalar_tensor_tensor(
                    ot[:], pwt[:], wbe[ob], xs[ob][:, lo:lo + 512],
                    OP.add, OP.add)
                (dma_eng or nc.sync).dma_start(
                    out_d[ob * 128:(ob + 1) * 128, lo:lo + 512], ot[:])

            # Z accumulation on DVE (bf16 2x mode): binary-counter tree
            # for j < ZRUN_J, then in-place running adds.
            def tree_push(q, lvl, t_node):
                st = state[q]
                pend = st["pend"]
                if pend.get(lvl) is None:
                    pend[lvl] = t_node
                    return
                a, b = pend.pop(lvl), t_node
                out = ztpool.tile([128, QW], BF16, tag=f"l{lvl}",
                                  name=f"l{lvl}_{q}")
                nc.vector.tensor_add(out[:], a[:], b[:])
                tree_push(q, lvl + 1, out)

            def z_accum(q, j, ef):
                st = state[q]
                if j < ZRUN_J:
                    if j % 2 == 1:
                        pair = ztpool.tile([128, QW], BF16, tag="l0",
                                           name=f"l0_{q}_{j}")
                        nc.vector.tensor_add(pair[:], efs[q * JB + j - 1][:],
                                             ef[:])
                        tree_push(q, 1, pair)
                elif j == ZRUN_J:
                    # merge pending counter partials (sum of 0..23), then run
                    zq = st["zq"] = ztpool.tile([128, QW], BF16, tag="zq",
                                                name=f"zq_{q}")
                    p4, p3 = st["pend"].pop(4), st["pend"].pop(3)
                    nc.vector.tensor_add(zq[:], p4[:], p3[:])
                    nc.vector.tensor_add(zq[:], zq[:], ef[:])
                elif q < NQ - 1 or j < JB - 4:
                    nc.vector.tensor_add(st["zq"][:], st["zq"][:], ef[:])
                else:
                    # last 4 adds of the final quarter in halves: s0 of zq
                    # completes right after the last exp so the tail's
                    # Z-close (subtile dep) starts ~0.5us earlier
                    for s in range(2):
                        sl = slice(s * 512, (s + 1) * 512)
                        nc.vector.tensor_add(st["zq"][:, sl], st["zq"][:, sl],
                                             ef[:, sl])

            def emit_exp(q, j, pft, ef):
                if (q, j) in DVE_SET:
                    nc.vector.tensor_scalar(
                        ef[:].bitcast(I16), pft[:],
                        float(CB), 0.0, OP.add, OP.max)
                else:
                    nc.scalar.activation(ef[:], pft[:], AF.Exp,
                                         bias=aux[:, 5:6], scale=float(SCL))

            def emit_f(t, pft, s):
                q, j = divmod(t, JB)
                i0 = q * QW
                nc.tensor.matmul(
                    pft[:, s * 512:(s + 1) * 512],
                    ph_sb[:, j * 128:(j + 1) * 128],
                    th_sb[:, i0 + s * 512:i0 + (s + 1) * 512],
                    start=True, stop=True)

            def emit_y(ty):
                qy, jy = divmod(ty, JB)
                if jy == 0:
                    state[qy]["pyt"] = py.tile([128, QW], F32, tag="py",
                                               name=f"py_{qy}")
                efy = efs.pop(ty)
                for s in range(2):
                    nc.tensor.matmul(
                        state[qy]["pyt"][:, s * 512:(s + 1) * 512],
                        gT_sb[:, jy * 128:(jy + 1) * 128],
                        efy[:, s * 512:(s + 1) * 512],
                        start=(jy == 0), stop=(jy == JB - 1))

            # ---------------- pre-loop: th0 / ph0 / th1, split t=0,1 ------
            # th0 as 2x256-col pieces (x arrives in 256-col pieces),
            # ph block 0 alone (gates f(0)), then ph blocks 1-3, th1.
            th_piece(0, 256, ev="s", pool_tag="pf")
            th_piece(256, 256, ev="v", pool_tag="pf")
            ph_piece(0, 128, ev="v", pool_tag="pf")
            ph_piece(128, 384, ev="v", pool_tag="pf")

            state[0] = {"pzt": [None, None], "zbt": [None, None], "pend": {}}
            pf0 = pf.tile([128, QW], F32, tag="pf", name="pf_0")
            pf1 = pf.tile([128, QW], F32, tag="pf", name="pf_1")
            ef0 = efpool.tile([128, QW], BF16, tag="ef", name="ef_0")
            ef1 = efpool.tile([128, QW], BF16, tag="ef", name="ef_1")
            efs[0], efs[1] = ef0, ef1

            emit_f(0, pf0, 0)
            emit_f(1, pf1, 0)
            nc.scalar.activation(ef0[:, 0:512], pf0[:, 0:512], AF.Exp,
                                 bias=aux[:, 5:6], scale=float(SCL))
            nc.scalar.activation(ef1[:, 0:512], pf1[:, 0:512], AF.Exp,
                                 bias=aux[:, 5:6], scale=float(SCL))
            # s1 halves gated on th chunk 1 (x cols 512:1024); pw pool —
            # a pf-pool tile here would cycle with pf_0's release (which
            # waits on exp0b, which waits on this very piece).
            th_piece(512, 512, ev="v", pool_tag="pw")
            emit_f(0, pf0, 1)
            emit_f(1, pf1, 1)
            nc.scalar.activation(ef0[:, 512:1024], pf0[:, 512:1024], AF.Exp,
                                 bias=aux[:, 5:6], scale=float(SCL))
            nc.scalar.activation(ef1[:, 512:1024], pf1[:, 512:1024], AF.Exp,
                                 bias=aux[:, 5:6], scale=float(SCL))

            # sched ops of t<2 (the main loop starts at t=2): ph chunk 1
            for op in sched.get(0, []) + sched.get(1, []):
                emit_sched_op(op)

            # ---------------- main flat pipeline ----------------
            y_ptr = 0
            for t in range(2, T):
                q, j = divmod(t, JB)
                if j == 0:
                    state[q] = {"pzt": [None, None], "zbt": [None, None],
                                "pend": {}}
                st = state[q]
                pft = pf.tile([128, QW], F32, tag="pf", name=f"pf_{t}")
                emit_f(t, pft, 0)
                emit_f(t, pft, 1)
                ef = efpool.tile([128, QW], BF16, tag="ef", name=f"ef_{t}")
                efs[t] = ef
                emit_exp(q, j, pft, ef)
                if t == 3:
                    z_accum(0, 1, efs[1])   # deferred j=1 pair (split exps)
                z_accum(q, j, ef)
                # previous quarter's deferred work
                if q > 0:
                    if j == ZCLOSE_J[0]:
                        zclose(q - 1, 0)
                    elif j == ZCLOSE_J[1]:
                        zclose(q - 1, 1)
                    elif j == ZINV_J:
                        zinv(q - 1, 0)
                        zinv(q - 1, 1)
                    elif j == BCAST_J[0]:
                        bcast_pe(q - 1, 0)
                    elif j == BCAST_J[1]:
                        bcast_pe(q - 1, 1)
                    elif j == MULT_J[0]:
                        mult(q - 1, 0)
                    elif j == MULT_J[1]:
                        mult(q - 1, 1)
                    elif WPROJ_J0 <= j < WPROJ_J0 + 4:
                        wproj(q - 1, (0, 2, 1, 3)[j - WPROJ_J0])
                for op in sched.get(t, []):
                    emit_sched_op(op)
                # trailing y accumulation (with q3 lag rampdown)
                if t - YLAG >= 0 and y_ptr <= t - YLAG:
                    emit_y(y_ptr)
                    y_ptr += 1
                if q == NQ - 1 and j in YEXTRA and y_ptr <= t - 2:
                    emit_y(y_ptr)
                    y_ptr += 1

            # ---------------- last quarter's tail (pipelined by half) ------
            q = NQ - 1
            pending = list(range(y_ptr, T))
            # all pending s0 y-MMs first so py s0 closes early, then
            # zclose(s0) can slot in before the s1 drain.
            for ty in pending:
                qy, jy = divmod(ty, JB)
                efy = efs[ty]
                nc.tensor.matmul(
                    state[qy]["pyt"][:, 0:512],
                    gT_sb[:, jy * 128:(jy + 1) * 128],
                    efy[:, 0:512],
                    start=(jy == 0), stop=(jy == JB - 1))
            zclose(q, 0)
            for ty in pending:
                qy, jy = divmod(ty, JB)
                efy = efs.pop(ty)
                nc.tensor.matmul(
                    state[qy]["pyt"][:, 512:1024],
                    gT_sb[:, jy * 128:(jy + 1) * 128],
                    efy[:, 512:1024],
                    start=(jy == 0), stop=(jy == JB - 1))
            zclose(q, 1)
            zinv(q, 0)
            zinv(q, 1)
            bcast_pe(q, 0)
            mult(q, 0)
            wproj(q, 0, nc.scalar)
            bcast_pe(q, 1)
            wproj(q, 2)
            mult(q, 1)
            wproj(q, 1, nc.scalar)
            wproj(q, 3)

    nc.compile()
    return nc


_CACHE = {}


def _get_nc():
    if "nc" not in _CACHE:
        _CACHE["nc"] = build()
    return _CACHE["nc"]


def _in_maps(x, g_w, g_b, theta_w, theta_b, phi_w, phi_b, W_w, W_b):
    x = np.ascontiguousarray(np.asarray(x, dtype=np.float32))
    wbe = (np.asarray(W_w, np.float32) @ np.asarray(g_b, np.float32)
           + np.asarray(W_b, np.float32))
    common = {
        "thw_t": np.ascontiguousarray(np.asarray(theta_w, np.float32).T),
        "phw_t": np.ascontiguousarray(np.asarray(phi_w, np.float32).T),
        "gw_t": np.ascontiguousarray(np.asarray(g_w, np.float32).T),
        "ww_t": np.ascontiguousarray(np.asarray(W_w, np.float32).T),
        "aux": np.stack(
            [
                np.asarray(theta_b, np.float32) * np.float32(A_SCALE),
                np.asarray(phi_b, np.float32),
                wbe[:128],
                wbe[128:],
                np.ones(128, np.float32),
                np.full(128, EXP_BIAS, np.float32),
            ],
            axis=1,
        ),
    }
    return [
        {"x": np.ascontiguousarray(x[b].reshape(C, N)), **common}
        for b in range(B)
    ]


def run(in_maps, **kw):
    nc = _get_nc()
    return run_bass_kernel_spmd(nc, in_maps, list(range(B)), **kw)


def kernel(**inputs):
    res = run(_in_maps(**inputs))
    out = np.stack([res.results[b]["out"] for b in range(B)])
    return out.reshape(B, C, H, Wd)


# revision 16
# speedup vs baseline: 1.0218x; 1.0218x over previous
"""NonLocalBlock (embedded-gaussian attention) TRN2 kernel, v3.

Shapes (hardcoded): x [8, 256, 64, 64] fp32, one batch element per core.
Per core:
  theta' = A * (theta_w^T x + theta_b)   (A = 128*log2(e); f' = A*f)
  phi/g  = 1x1 conv projections, [128, 4096]
  f'^T[j, i] = sum_c phi[c, j] theta'[c, i]   (A-scaled 4096x4096 logits)
  Scalar tiles: ef = Exp(f' * (1/A) + (-10))   (global bias -10)
  DVE tiles (9 of 128): Schraudolph bf16 fast-exp in ONE tensor_scalar:
      bits_i16 = trunc(max(f' + CB, 0)), viewed as bf16 ~= e^(f-10)
      (CB = 16251 - A*10; piecewise-linear 2^frac, +-3.3% per elem,
       validated 2.3e-3 rel err end-to-end vs 2e-2 gate)
  Z[i] = sum_j ef[j, i]  via bf16 DVE adds (binary tree then running),
         closed by ones-matmul partition reduce; 1/Z via fast reciprocal;
         Z broadcast via PE ones-row matmul into PSUM (not gpsimd: its
         tensor ops share the DVE SBUF port and inflate DVE ops ~2x)
  y[ci, i] = sum_j ef[j, i] gT[j, ci]   (bf16 matmuls, fp32 PSUM)
  out = x + W_w @ (y * (1/Z)) + wbe,  wbe = W_w @ g_b + W_b folded into
        the W-proj eviction via scalar_tensor_tensor (kills the 16
        bias-add DVE ops of v2)

Engine budget per t (cadence target ~1.03us): Scalar 119 exps x 1.08;
DVE Z-add 0.67 + evictions/mult/STT + 9 fast-exps x 1.24; PE f 0.43 +
y 0.43 + proj/wproj/zclose/bcast ~0.12.

Ramp: 3 DMA queues (sync/scalar hw + gpsimd swdge for weights), first x
pieces 256-col, th0 in 2x256 pieces, t=0/1 exps split in halves so the
Scalar pipe starts ~13.2us instead of 21us.

Tail: y-lag ramps 11->5 over Q3 (extra y slots), then half-pipelined
zclose->zinv->bcastPE->mult->wproj chains alternating DMA queues.
"""

import numpy as np

import concourse.bacc as bacc
import concourse.mybir as mybir
from concourse import tile
from concourse.bass_utils import run_bass_kernel_spmd

F32 = mybir.dt.float32
F32R = mybir.dt.float32r
BF16 = mybir.dt.bfloat16
I16 = mybir.dt.int16
AF = mybir.ActivationFunctionType
OP = mybir.AluOpType

B, C, CI = 8, 256, 128
H, Wd = 64, 64
N = H * Wd              # 4096
NQ = 4                  # i-quarters
QW = N // NQ            # 1024
JB = N // 128           # 32 j-blocks (= ts per quarter)
T = NQ * JB             # 128

YLAG = 11               # steady-state y-matmul lag behind f/exp
YEXTRA = {17, 19, 21, 23, 25, 27}   # q3 slots emitting a 2nd y (lag 11->5)
ZCLOSE_J = (3, 4)       # prev quarter's Z partition-reduce MMs
ZINV_J = 5              # prev quarter's reciprocals
BCAST_J = (9, 10)       # prev quarter's Z broadcast (PE ones-row MM)
MULT_J = (11, 12)       # prev quarter's normalize halves
WPROJ_J0 = 13           # prev quarter's W-projection chunks j=13..16
ZRUN_J = 24             # switch Z accumulation from tree to running adds

EXP_BIAS = -10.0        # global logit shift (f range [-90.8, 84.8])
A_SCALE = 128.0 * 1.4426950408889634   # 184.66496...
CB = np.float32(16251.0 + A_SCALE * EXP_BIAS)  # schraudolph add const
SCL = np.float32(1.0 / A_SCALE)
NWARM = 6

# (q, j) tiles whose exp runs on DVE (fast-exp); chosen away from
# deferred-op j slots and sched-heavy ts.  k=9.
DVE_SET = {(1, 20), (1, 28),
           (2, 8), (2, 20), (2, 28),
           (3, 2), (3, 8), (3, 20), (3, 26)}


def _build_sched():
    # t -> list of ops. pieces are 512-col units c=0..7 of x columns.
    # deadlines: ph piece c before f uses j-block 4c (t=4c); th piece c
    # before f of quarter c//2 (t=32*(c//2)); g piece c before y uses
    # block 4c (t=YLAG+4c).  j in {9..16} of q>=1 avoided (pw pool is
    # busy with zb/wproj there).
    sched = {}

    def add(t, op):
        sched.setdefault(t, []).append(op)

    for c in range(1, 8):
        add(4 * c - 3, ("ph", c))
    for c, t in ((2, 24), (3, 26), (4, 50), (5, 54), (6, 82), (7, 86)):
        add(t, ("th", c))
    gsched = {0: 2, 1: 6, 2: 10, 3: 14, 4: 18, 5: 22, 6: 30, 7: 33}
    for c, t in gsched.items():
        add(t, ("ga", c))
        add(t + 1, ("gb", c))
    return sched


def build():
    nc = bacc.Bacc("TRN2", target_bir_lowering=False, debug=False, num_devices=8)

    x_d = nc.dram_tensor("x", [C, N], F32R, kind="ExternalInput")
    thw_d = nc.dram_tensor("thw_t", [C, CI], F32R, kind="ExternalInput")  # theta_w.T
    phw_d = nc.dram_tensor("phw_t", [C, CI], F32R, kind="ExternalInput")  # phi_w.T
    gw_d = nc.dram_tensor("gw_t", [C, CI], F32R, kind="ExternalInput")    # g_w.T
    ww_d = nc.dram_tensor("ww_t", [CI, C], F32R, kind="ExternalInput")    # W_w.T
    # aux cols: 0=A*theta_b, 1=phi_b, 2=wbe[:128], 3=wbe[128:], 4=ones,
    # 5=exp bias (-10)
    aux_d = nc.dram_tensor("aux", [128, 6], F32, kind="ExternalInput")
    out_d = nc.dram_tensor("out", [C, N], F32, kind="ExternalOutput")

    sched = _build_sched()

    with tile.TileContext(nc) as tc:
        with (
            tc.tile_pool(name="const", bufs=1) as cpool,
            tc.tile_pool(name="big", bufs=1) as bigpool,
            tc.tile_pool(name="ef", bufs=13) as efpool,
            tc.tile_pool(name="ztree", bufs=2) as ztpool,
            tc.tile_pool(name="zpool", bufs=2) as zpool,
            tc.tile_pool(name="ypool", bufs=2) as ypool,
            tc.tile_pool(name="opool", bufs=6) as opool,
            tc.tile_pool(name="pf", bufs=2, space="PSUM") as pf,
            tc.tile_pool(name="py", bufs=1, space="PSUM") as py,
            tc.tile_pool(name="pw", bufs=2, space="PSUM") as pw,
        ):
            # ---------------- warmup + DMA issue ----------------
            warm = cpool.tile([128, 512], BF16, tag="warm")
            warm2 = cpool.tile([128, 1], F32, tag="warm2")
            nc.gpsimd.memset(warm[:], 0.0)

            aux = cpool.tile([128, 6], F32, tag="aux")
            thw = cpool.tile([128, 2 * CI], F32R, tag="thw")
            phw = cpool.tile([128, 2 * CI], F32R, tag="phw")
            gw = cpool.tile([128, 2 * CI], F32R, tag="gw")
            ww = cpool.tile([CI, C], F32R, tag="ww")
            x0 = bigpool.tile([128, N], F32R, tag="x0")
            x1 = bigpool.tile([128, N], F32R, tag="x1")
            xs = (x0, x1)

            # x on the two hw queues (256-col first pieces: the rings
            # ramp slowly and the first piece gates the first proj);
            # all weights on the gpsimd software-DGE queue.
            nc.sync.dma_start(aux[:], aux_d[:])
            nc.sync.dma_start(x0[:, 0:256], x_d[0:128, 0:256])
            nc.scalar.dma_start(x1[:, 0:256], x_d[128:256, 0:256])
            nc.sync.dma_start(thw[:, 0:CI], thw_d[0:128, :])
            nc.scalar.dma_start(thw[:, CI:2 * CI], thw_d[128:256, :])
            # dummy activation pulls the ~1.3us exp table load early
            nc.scalar.activation(warm2[:], warm[:, 0:1], AF.Identity)
            nc.sync.dma_start(x0[:, 256:512], x_d[0:128, 256:512])
            nc.scalar.dma_start(x1[:, 256:512], x_d[128:256, 256:512])
            nc.sync.dma_start(phw[:, 0:CI], phw_d[0:128, :])
            nc.scalar.dma_start(phw[:, CI:2 * CI], phw_d[128:256, :])
            nc.sync.dma_start(x0[:, 512:1024], x_d[0:128, 512:1024])
            nc.scalar.dma_start(x1[:, 512:1024], x_d[128:256, 512:1024])
            nc.sync.dma_start(gw[:, 0:CI], gw_d[0:128, :])
            nc.scalar.dma_start(gw[:, CI:2 * CI], gw_d[128:256, :])
            nc.sync.dma_start(x0[:, 1024:2048], x_d[0:128, 1024:2048])
            nc.scalar.dma_start(x1[:, 1024:2048], x_d[128:256, 1024:2048])
            nc.sync.dma_start(ww[:], ww_d[:])
            nc.sync.dma_start(x0[:, 2048:4096], x_d[0:128, 2048:4096])
            nc.scalar.dma_start(x1[:, 2048:4096], x_d[128:256, 2048:4096])

            for _ in range(NWARM):
                pwt = pw.tile([128, 512], F32, tag="pw", name="warm_mm")
                nc.tensor.matmul(pwt[:], warm[:, 0:128], warm[:],
                                 start=True, stop=True)

            thbA = aux[:, 0:1]   # A * theta_b
            phb = aux[:, 1:2]
            wbe = (aux[:, 2:3], aux[:, 3:4])
            ones_bf = cpool.tile([128, 1], BF16, tag="ones_bf")
            nc.vector.tensor_copy(ones_bf[:], aux[:, 4:5])

            th_sb = bigpool.tile([128, N], F32R, tag="th")
            ph_sb = bigpool.tile([128, N], F32R, tag="ph")
            gT_sb = bigpool.tile([128, N], BF16, tag="gT")

            # ---------------- projection piece emitters ----------------
            def proj_mm(pp, wt, lo, w):
                for k in range(2):
                    nc.tensor.matmul(
                        pp[:, 0:w], wt[:, k * CI:(k + 1) * CI],
                        xs[k][:, lo:lo + w],
                        start=(k == 0), stop=(k == 1),
                    )

            def th_piece(lo, w, ev="v", pool_tag="pw"):
                # theta' = A*(proj + thb): evict applies scale A + A*thb
                pool = pf if pool_tag == "pf" else pw
                pp = pool.tile([128, 512], F32, tag=pool_tag,
                               name=f"thp_{lo}")
                proj_mm(pp, thw, lo, w)
                if ev == "s":
                    nc.scalar.activation(th_sb[:, lo:lo + w], pp[:, 0:w],
                                         AF.Identity, bias=thbA,
                                         scale=float(A_SCALE))
                else:
                    nc.vector.tensor_scalar(
                        th_sb[:, lo:lo + w], pp[:, 0:w],
                        float(A_SCALE), thbA, OP.mult, OP.add)

            def ph_piece(lo, w, ev="v", pool_tag="pw"):
                pool = pf if pool_tag == "pf" else pw
                pp = pool.tile([128, 512], F32, tag=pool_tag,
                               name=f"php_{lo}")
                proj_mm(pp, phw, lo, w)
                if ev == "s":
                    nc.scalar.activation(ph_sb[:, lo:lo + w], pp[:, 0:w],
                                         AF.Identity, bias=phb)
                else:
                    nc.vector.tensor_scalar_add(ph_sb[:, lo:lo + w],
                                                pp[:, 0:w], phb)

            gtiles = {}

            def g_piece(c, half):
                # gT blocks 4c+2*half, 4c+2*half+1 into shared [128,512] tile
                if half == 0:
                    gtiles[c] = pw.tile([128, 512], F32, tag="pw",
                                        name=f"gp_{c}")
                pg = gtiles[c]
                for b in (4 * c + 2 * half, 4 * c + 2 * half + 1):
                    col = (b - 4 * c) * 128
                    for k in range(2):
                        nc.tensor.matmul(
                            pg[:, col:col + 128],
                            xs[k][:, b * 128:(b + 1) * 128],
                            gw[:, k * CI:(k + 1) * CI],
                            start=(k == 0), stop=(k == 1),
                        )
                if half == 1:
                    lo = c * 512
                    nc.vector.tensor_copy(gT_sb[:, lo:lo + 512], pg[:])
                    del gtiles[c]

            def emit_sched_op(op):
                kind = op[0]
                if kind == "th":
                    th_piece(op[1] * 512, 512)
                elif kind == "ph":
                    ph_piece(op[1] * 512, 512)
                elif kind == "ga":
                    g_piece(op[1], 0)
                elif kind == "gb":
                    g_piece(op[1], 1)

            # ---------------- per-quarter deferred ops ----------------
            state = {}   # per-quarter: zq, pzt[2], zi, zbt[2], pyt, ynt
            efs = {}

            def zclose(q, s):
                st = state[q]
                pzt = pw.tile([1, 512], F32, tag="pw", name=f"pz_{q}_{s}")
                st["pzt"][s] = pzt
                nc.tensor.matmul(pzt[:], ones_bf[:],
                                 st["zq"][:, s * 512:(s + 1) * 512],
                                 start=True, stop=True)

            def zinv(q, s):
                # fast variant: ~18 correct bits; Z is far from the edges.
                # zi is F32R so the bcast matmul can consume it at 1 cyc/row
                # (the BIR verifier requires f32r matmul inputs to be
                # *written* as f32r); _custom_dve directly since the
                # wrapper asserts fp32 out.
                from concourse.dve_ops import (
                    RECIP_APPROX_FAST_CONSTS,
                    RECIPROCAL_APPROX_FAST,
                )
                st = state[q]
                if s == 0:
                    st["zi"] = zpool.tile([1, QW], F32R, tag="zi",
                                          name=f"zi_{q}")
                c = RECIP_APPROX_FAST_CONSTS
                nc.vector._custom_dve(
                    RECIPROCAL_APPROX_FAST,
                    out=st["zi"][:, s * 512:(s + 1) * 512],
                    in0=st["pzt"][s][:],
                    s0=c["s0"], s1=c["s1"], imm2=c["imm2"])

            def bcast_pe(q, s):
                # gpsimd partition broadcast (DVE tensor_tensor cannot read
                # two PSUM operands, so zb must live in SBUF; PE can only
                # write PSUM).  gpsimd is otherwise idle.
                st = state[q]
                if s == 0:
                    st["zb"] = zpool.tile([128, QW], F32, tag="zb",
                                          name=f"zb_{q}")
                nc.gpsimd.partition_broadcast(
                    st["zb"][:, s * 512:(s + 1) * 512],
                    st["zi"][:, s * 512:(s + 1) * 512].bitcast(F32))

            def mult(q, s):
                st = state[q]
                if s == 0:
                    st["ynt"] = ypool.tile([128, QW], F32R, tag="ynt",
                                           name=f"ynt_{q}")
                nc.vector.tensor_mul(
                    st["ynt"][:, s * 512:(s + 1) * 512],
                    st["pyt"][:, s * 512:(s + 1) * 512],
                    st["zb"][:, s * 512:(s + 1) * 512])

            def wproj(q, chunk, dma_eng=None):
                ob, s2 = divmod(chunk, 2)
                lo = q * QW + s2 * 512
                pwt = pw.tile([128, 512], F32, tag="pw",
                              name=f"pw_{q}_{chunk}")
                nc.tensor.matmul(
                    pwt[:], ww[:, ob * CI:(ob + 1) * CI],
                    state[q]["ynt"][:, s2 * 512:(s2 + 1) * 512],
                    start=True, stop=True)
                ot = opool.tile([128, 512], F32, tag="o", name=f"o_{q}_{chunk}")
                # out = (W@y/Z + wbe) + x  in one DVE op
                nc.vector.scalar_tensor_tensor(
                    ot[:], pwt[:], wbe[ob], xs[ob][:, lo:lo + 512],
                    OP.add, OP.add)
                (dma_eng or nc.sync).dma_start(
                    out_d[ob * 128:(ob + 1) * 128, lo:lo + 512], ot[:])

            # Z accumulation on DVE (bf16 2x mode): binary-counter tree
            # for j < ZRUN_J, then in-place running adds.
            def tree_push(q, lvl, t_node):
                st = state[q]
                pend = st["pend"]
                if pend.get(lvl) is None:
                    pend[lvl] = t_node
                    return
                a, b = pend.pop(lvl), t_node
                out = ztpool.tile([128, QW], BF16, tag=f"l{lvl}",
                                  name=f"l{lvl}_{q}")
                nc.vector.tensor_add(out[:], a[:], b[:])
                tree_push(q, lvl + 1, out)

            def z_accum(q, j, ef):
                st = state[q]
                if j < ZRUN_J:
                    if j % 2 == 1:
                        pair = ztpool.tile([128, QW], BF16, tag="l0",
                                           name=f"l0_{q}_{j}")
                        nc.vector.tensor_add(pair[:], efs[q * JB + j - 1][:],
                                             ef[:])
                        tree_push(q, 1, pair)
                elif j == ZRUN_J:
                    # merge pending counter partials (sum of 0..23), then run
                    zq = st["zq"] = ztpool.tile([128, QW], BF16, tag="zq",
                                                name=f"zq_{q}")
                    p4, p3 = st["pend"].pop(4), st["pend"].pop(3)
                    nc.vector.tensor_add(zq[:], p4[:], p3[:])
                    nc.vector.tensor_add(zq[:], zq[:], ef[:])
                elif q < NQ - 1 or j < JB - 4:
                    nc.vector.tensor_add(st["zq"][:], st["zq"][:], ef[:])
                else:
                    # last 4 adds of the final quarter in halves: s0 of zq
                    # completes right after the last exp so the tail's
                    # Z-close (subtile dep) starts ~0.5us earlier
                    for s in range(2):
                        sl = slice(s * 512, (s + 1) * 512)
                        nc.vector.tensor_add(st["zq"][:, sl], st["zq"][:, sl],
                                             ef[:, sl])

            def emit_dve_tile(t, ef):
                # DVE fast-exp tiles get their own pw-pool PSUM halves so
                # the slow (1.2us) DVE read never stalls the pf rotation
                # that feeds the Scalar exp pipe (measured ~2us bubbles).
                q, j = divmod(t, JB)
                i0 = q * QW
                for s in range(2):
                    pp = pw.tile([128, 512], F32, tag="pw",
                                 name=f"fd_{t}_{s}")
                    nc.tensor.matmul(
                        pp[:], ph_sb[:, j * 128:(j + 1) * 128],
                        th_sb[:, i0 + s * 512:i0 + (s + 1) * 512],
                        start=True, stop=True)
                    nc.vector.tensor_scalar(
                        ef[:, s * 512:(s + 1) * 512].bitcast(I16), pp[:],
                        float(CB), 0.0, OP.add, OP.max)

            def emit_f(t, pft, s):
                q, j = divmod(t, JB)
                i0 = q * QW
                nc.tensor.matmul(
                    pft[:, s * 512:(s + 1) * 512],
                    ph_sb[:, j * 128:(j + 1) * 128],
                    th_sb[:, i0 + s * 512:i0 + (s + 1) * 512],
                    start=True, stop=True)

            def emit_y(ty):
                qy, jy = divmod(ty, JB)
                if jy == 0:
                    state[qy]["pyt"] = py.tile([128, QW], F32, tag="py",
                                               name=f"py_{qy}")
                efy = efs.pop(ty)
                for s in range(2):
                    nc.tensor.matmul(
                        state[qy]["pyt"][:, s * 512:(s + 1) * 512],
                        gT_sb[:, jy * 128:(jy + 1) * 128],
                        efy[:, s * 512:(s + 1) * 512],
                        start=(jy == 0), stop=(jy == JB - 1))

            # ---------------- pre-loop: th0 / ph0 / th1, split t=0,1 ------
            # PE-queue order matters (in-order engine): f0s0 is emitted as
            # soon as its gates (th0a/th0b/ph0a) are, so exp0a isn't stuck
            # behind ph0b/th1 matmuls.
            state[0] = {"pzt": [None, None], "zbt": [None, None], "pend": {}}
            pf0 = pf.tile([128, QW], F32, tag="pf", name="pf_0")
            pf1 = pf.tile([128, QW], F32, tag="pf", name="pf_1")
            ef0 = efpool.tile([128, QW], BF16, tag="ef", name="ef_0")
            ef1 = efpool.tile([128, QW], BF16, tag="ef", name="ef_1")
            efs[0], efs[1] = ef0, ef1

            th_piece(0, 256, ev="s", pool_tag="pw")
            th_piece(256, 256, ev="v", pool_tag="pw")
            ph_piece(0, 128, ev="v", pool_tag="pw")
            emit_f(0, pf0, 0)
            nc.scalar.activation(ef0[:, 0:512], pf0[:, 0:512], AF.Exp,
                                 bias=aux[:, 5:6], scale=float(SCL))
            ph_piece(128, 384, ev="v", pool_tag="pw")
            emit_f(1, pf1, 0)
            nc.scalar.activation(ef1[:, 0:512], pf1[:, 0:512], AF.Exp,
                                 bias=aux[:, 5:6], scale=float(SCL))
            # s1 halves gated on th chunk 1 (x cols 512:1024)
            th_piece(512, 512, ev="v", pool_tag="pw")
            emit_f(0, pf0, 1)
            emit_f(1, pf1, 1)
            nc.scalar.activation(ef0[:, 512:1024], pf0[:, 512:1024], AF.Exp,
                                 bias=aux[:, 5:6], scale=float(SCL))
            nc.scalar.activation(ef1[:, 512:1024], pf1[:, 512:1024], AF.Exp,
                                 bias=aux[:, 5:6], scale=float(SCL))

            # sched ops of t<2 (the main loop starts at t=2): ph chunk 1
            for op in sched.get(0, []) + sched.get(1, []):
                emit_sched_op(op)

            # ---------------- main flat pipeline ----------------
            y_ptr = 0
            for t in range(2, T):
                q, j = divmod(t, JB)
                if j == 0:
                    state[q] = {"pzt": [None, None], "zbt": [None, None],
                                "pend": {}}
                st = state[q]
                ef = efpool.tile([128, QW], BF16, tag="ef", name=f"ef_{t}")
                efs[t] = ef
                if (q, j) in DVE_SET:
                    emit_dve_tile(t, ef)
                else:
                    pft = pf.tile([128, QW], F32, tag="pf", name=f"pf_{t}")
                    emit_f(t, pft, 0)
                    emit_f(t, pft, 1)
                    nc.scalar.activation(ef[:], pft[:], AF.Exp,
                                         bias=aux[:, 5:6], scale=float(SCL))
                if t == 3:
                    z_accum(0, 1, efs[1])   # deferred j=1 pair (split exps)
                z_accum(q, j, ef)
                # previous quarter's deferred work
                if q > 0:
                    if j == ZCLOSE_J[0]:
                        zclose(q - 1, 0)
                    elif j == ZCLOSE_J[1]:
                        zclose(q - 1, 1)
                    elif j == ZINV_J:
                        zinv(q - 1, 0)
                        zinv(q - 1, 1)
                    elif j == BCAST_J[0]:
                        bcast_pe(q - 1, 0)
                    elif j == BCAST_J[1]:
                        bcast_pe(q - 1, 1)
                    elif j == MULT_J[0]:
                        mult(q - 1, 0)
                    elif j == MULT_J[1]:
                        mult(q - 1, 1)
                    elif WPROJ_J0 <= j < WPROJ_J0 + 4:
                        wproj(q - 1, (0, 2, 1, 3)[j - WPROJ_J0])
                for op in sched.get(t, []):
                    emit_sched_op(op)
                # trailing y accumulation (with q3 lag rampdown)
                if t - YLAG >= 0 and y_ptr <= t - YLAG:
                    emit_y(y_ptr)
                    y_ptr += 1
                if q == NQ - 1 and j in YEXTRA and y_ptr <= t - 2:
                    emit_y(y_ptr)
                    y_ptr += 1

            # ---------------- last quarter's tail (pipelined by half) ------
            q = NQ - 1
            pending = list(range(y_ptr, T))

            def y_half(ty, s):
                qy, jy = divmod(ty, JB)
                nc.tensor.matmul(
                    state[qy]["pyt"][:, s * 512:(s + 1) * 512],
                    gT_sb[:, jy * 128:(jy + 1) * 128],
                    efs[ty][:, s * 512:(s + 1) * 512],
                    start=(jy == 0), stop=(jy == JB - 1))

            # s0 y-drain, then the Z chains for both halves as early as
            # their deps allow (zq-s halves close right after the last
            # exp), with the s1 y-drain interleaved behind zclose(s1).
            for ty in pending:
                y_half(ty, 0)
            zclose(q, 0)
            y_half(pending[0], 1)
            y_half(pending[1], 1)
            zclose(q, 1)
            zinv(q, 0)
            zinv(q, 1)
            bcast_pe(q, 0)
            bcast_pe(q, 1)
            for ty in pending[2:]:
                y_half(ty, 1)
            mult(q, 0)
            wproj(q, 0, nc.scalar)
            wproj(q, 2)
            mult(q, 1)
            wproj(q, 1, nc.scalar)
            wproj(q, 3)

    nc.compile()
    return nc


_CACHE = {}


def _get_nc():
    if "nc" not in _CACHE:
        _CACHE["nc"] = build()
    return _CACHE["nc"]


def _in_maps(x, g_w, g_b, theta_w, theta_b, phi_w, phi_b, W_w, W_b):
    x = np.ascontiguousarray(np.asarray(x, dtype=np.float32))
    wbe = (np.asarray(W_w, np.float32) @ np.asarray(g_b, np.float32)
           + np.asarray(W_b, np.float32))
    common = {
        "thw_t": np.ascontiguousarray(np.asarray(theta_w, np.float32).T),
        "phw_t": np.ascontiguousarray(np.asarray(phi_w, np.float32).T),
        "gw_t": np.ascontiguousarray(np.asarray(g_w, np.float32).T),
        "ww_t": np.ascontiguousarray(np.asarray(W_w, np.float32).T),
        "aux": np.stack(
            [
                np.asarray(theta_b, np.float32) * np.float32(A_SCALE),
                np.asarray(phi_b, np.float32),
                wbe[:128],
                wbe[128:],
                np.ones(128, np.float32),
                np.full(128, EXP_BIAS, np.float32),
            ],
            axis=1,
        ),
    }
    return [
        {"x": np.ascontiguousarray(x[b].reshape(C, N)), **common}
        for b in range(B)
    ]


def run(in_maps, **kw):
    nc = _get_nc()
    return run_bass_kernel_spmd(nc, in_maps, list(range(B)), **kw)


def kernel(**inputs):
    res = run(_in_maps(**inputs))
    out = np.stack([res.results[b]["out"] for b in range(B)])
    return out.reshape(B, C, H, Wd)


# revision 17
# speedup vs baseline: 1.0448x; 1.0225x over previous
"""NonLocalBlock (embedded-gaussian attention) TRN2 kernel, v3.

Shapes (hardcoded): x [8, 256, 64, 64] fp32, one batch element per core.
Per core:
  theta' = A * (theta_w^T x + theta_b)   (A = 128*log2(e); f' = A*f)
  phi/g  = 1x1 conv projections, [128, 4096]
  f'^T[j, i] = sum_c phi[c, j] theta'[c, i]   (A-scaled 4096x4096 logits)
  Scalar tiles: ef = Exp(f' * (1/A) + (-10))   (global bias -10)
  DVE tiles (9 of 128): Schraudolph bf16 fast-exp in ONE tensor_scalar:
      bits_i16 = trunc(max(f' + CB, 0)), viewed as bf16 ~= e^(f-10)
      (CB = 16251 - A*10; piecewise-linear 2^frac, +-3.3% per elem,
       validated 2.3e-3 rel err end-to-end vs 2e-2 gate)
  Z[i] = sum_j ef[j, i]  via bf16 DVE adds (binary tree then running),
         closed by ones-matmul partition reduce; 1/Z via fast reciprocal;
         Z broadcast via PE ones-row matmul into PSUM (not gpsimd: its
         tensor ops share the DVE SBUF port and inflate DVE ops ~2x)
  y[ci, i] = sum_j ef[j, i] gT[j, ci]   (bf16 matmuls, fp32 PSUM)
  out = x + W_w @ (y * (1/Z)) + wbe,  wbe = W_w @ g_b + W_b folded into
        the W-proj eviction via scalar_tensor_tensor (kills the 16
        bias-add DVE ops of v2)

Engine budget per t (cadence target ~1.03us): Scalar 119 exps x 1.08;
DVE Z-add 0.67 + evictions/mult/STT + 9 fast-exps x 1.24; PE f 0.43 +
y 0.43 + proj/wproj/zclose/bcast ~0.12.

Ramp: 3 DMA queues (sync/scalar hw + gpsimd swdge for weights), first x
pieces 256-col, th0 in 2x256 pieces, t=0/1 exps split in halves so the
Scalar pipe starts ~13.2us instead of 21us.

Tail: y-lag ramps 11->5 over Q3 (extra y slots), then half-pipelined
zclose->zinv->bcastPE->mult->wproj chains alternating DMA queues.
"""

import numpy as np

import concourse.bacc as bacc
import concourse.mybir as mybir
from concourse import tile
from concourse.bass_utils import run_bass_kernel_spmd

F32 = mybir.dt.float32
F32R = mybir.dt.float32r
BF16 = mybir.dt.bfloat16
I16 = mybir.dt.int16
AF = mybir.ActivationFunctionType
OP = mybir.AluOpType

B, C, CI = 8, 256, 128
H, Wd = 64, 64
N = H * Wd              # 4096
NQ = 4                  # i-quarters
QW = N // NQ            # 1024
JB = N // 128           # 32 j-blocks (= ts per quarter)
T = NQ * JB             # 128

YLAG = 11               # steady-state y-matmul lag behind f/exp
YEXTRA = {17, 19, 21, 23, 25, 27}   # q3 slots emitting a 2nd y (lag 11->5)
ZCLOSE_J = (3, 4)       # prev quarter's Z partition-reduce MMs
ZINV_J = 5              # prev quarter's reciprocals
BCAST_J = (9, 10)       # prev quarter's Z broadcast (PE ones-row MM)
MULT_J = (11, 12)       # prev quarter's normalize halves
WPROJ_J0 = 13           # prev quarter's W-projection chunks j=13..16
ZRUN_J = 24             # switch Z accumulation from tree to running adds

EXP_BIAS = -10.0        # global logit shift (f range [-90.8, 84.8])
A_SCALE = 128.0 * 1.4426950408889634   # 184.66496...
CB = np.float32(16251.0 + A_SCALE * EXP_BIAS)  # schraudolph add const
SCL = np.float32(1.0 / A_SCALE)
NWARM = 6

# (q, j) tiles whose exp runs on DVE (fast-exp); chosen away from
# deferred-op j slots and sched-heavy ts.  k=9.
DVE_SET = {(1, 6), (1, 20), (1, 26),
           (2, 2), (2, 6), (2, 20), (2, 26),
           (3, 2), (3, 8), (3, 18), (3, 22)}


def _build_sched():
    # t -> list of ops. pieces are 512-col units c=0..7 of x columns.
    # deadlines: ph piece c before f uses j-block 4c (t=4c); th piece c
    # before f of quarter c//2 (t=32*(c//2)); g piece c before y uses
    # block 4c (t=YLAG+4c).  j in {9..16} of q>=1 avoided (pw pool is
    # busy with zb/wproj there).
    sched = {}

    def add(t, op):
        sched.setdefault(t, []).append(op)

    for c in range(1, 8):
        add(4 * c - 3, ("ph", c))
    for c, t in ((2, 24), (3, 26), (4, 50), (5, 54), (6, 82), (7, 86)):
        add(t, ("th", c))
    gsched = {0: 2, 1: 6, 2: 10, 3: 14, 4: 18, 5: 22, 6: 30, 7: 33}
    for c, t in gsched.items():
        add(t, ("ga", c))
        add(t + 1, ("gb", c))
    return sched


def build():
    nc = bacc.Bacc("TRN2", target_bir_lowering=False, debug=False, num_devices=8)

    x_d = nc.dram_tensor("x", [C, N], F32R, kind="ExternalInput")
    thw_d = nc.dram_tensor("thw_t", [C, CI], F32R, kind="ExternalInput")  # theta_w.T
    phw_d = nc.dram_tensor("phw_t", [C, CI], F32R, kind="ExternalInput")  # phi_w.T
    gw_d = nc.dram_tensor("gw_t", [C, CI], F32R, kind="ExternalInput")    # g_w.T
    ww_d = nc.dram_tensor("ww_t", [CI, C], F32R, kind="ExternalInput")    # W_w.T
    # aux cols: 0=A*theta_b, 1=phi_b, 2=wbe[:128], 3=wbe[128:], 4=ones,
    # 5=exp bias (-10)
    aux_d = nc.dram_tensor("aux", [128, 6], F32, kind="ExternalInput")
    out_d = nc.dram_tensor("out", [C, N], F32, kind="ExternalOutput")

    sched = _build_sched()

    with tile.TileContext(nc) as tc:
        with (
            tc.tile_pool(name="const", bufs=1) as cpool,
            tc.tile_pool(name="big", bufs=1) as bigpool,
            tc.tile_pool(name="ef", bufs=13) as efpool,
            tc.tile_pool(name="ztree", bufs=2) as ztpool,
            tc.tile_pool(name="zpool", bufs=2) as zpool,
            tc.tile_pool(name="ypool", bufs=2) as ypool,
            tc.tile_pool(name="opool", bufs=6) as opool,
            tc.tile_pool(name="pf", bufs=2, space="PSUM") as pf,
            tc.tile_pool(name="py", bufs=1, space="PSUM") as py,
            tc.tile_pool(name="pw", bufs=2, space="PSUM") as pw,
        ):
            # ---------------- warmup + DMA issue ----------------
            warm = cpool.tile([128, 512], BF16, tag="warm")
            warm2 = cpool.tile([128, 1], F32, tag="warm2")
            nc.gpsimd.memset(warm[:], 0.0)

            aux = cpool.tile([128, 6], F32, tag="aux")
            thw = cpool.tile([128, 2 * CI], F32R, tag="thw")
            phw = cpool.tile([128, 2 * CI], F32R, tag="phw")
            gw = cpool.tile([128, 2 * CI], F32R, tag="gw")
            ww = cpool.tile([CI, C], F32R, tag="ww")
            x0 = bigpool.tile([128, N], F32R, tag="x0")
            x1 = bigpool.tile([128, N], F32R, tag="x1")
            xs = (x0, x1)

            # x on the two hw queues (256-col first pieces: the rings
            # ramp slowly and the first piece gates the first proj);
            # all weights on the gpsimd software-DGE queue.
            nc.sync.dma_start(aux[:], aux_d[:])
            nc.sync.dma_start(x0[:, 0:256], x_d[0:128, 0:256])
            nc.scalar.dma_start(x1[:, 0:256], x_d[128:256, 0:256])
            nc.sync.dma_start(thw[:, 0:CI], thw_d[0:128, :])
            nc.scalar.dma_start(thw[:, CI:2 * CI], thw_d[128:256, :])
            # dummy activation pulls the ~1.3us exp table load early
            nc.scalar.activation(warm2[:], warm[:, 0:1], AF.Identity)
            nc.sync.dma_start(x0[:, 256:512], x_d[0:128, 256:512])
            nc.scalar.dma_start(x1[:, 256:512], x_d[128:256, 256:512])
            nc.sync.dma_start(phw[:, 0:CI], phw_d[0:128, :])
            nc.scalar.dma_start(phw[:, CI:2 * CI], phw_d[128:256, :])
            nc.sync.dma_start(x0[:, 512:1024], x_d[0:128, 512:1024])
            nc.scalar.dma_start(x1[:, 512:1024], x_d[128:256, 512:1024])

            for _ in range(NWARM):
                pwt = pw.tile([128, 512], F32, tag="pw", name="warm_mm")
                nc.tensor.matmul(pwt[:], warm[:, 0:128], warm[:],
                                 start=True, stop=True)

            thbA = aux[:, 0:1]   # A * theta_b
            phb = aux[:, 1:2]
            wbe = (aux[:, 2:3], aux[:, 3:4])
            ones_bf = cpool.tile([128, 1], BF16, tag="ones_bf")
            nc.vector.tensor_copy(ones_bf[:], aux[:, 4:5])

            th_sb = bigpool.tile([128, N], F32R, tag="th")
            ph_sb = bigpool.tile([128, N], F32R, tag="ph")
            gT_sb = bigpool.tile([128, N], BF16, tag="gT")

            # ---------------- projection piece emitters ----------------
            def proj_mm(pp, wt, lo, w):
                for k in range(2):
                    nc.tensor.matmul(
                        pp[:, 0:w], wt[:, k * CI:(k + 1) * CI],
                        xs[k][:, lo:lo + w],
                        start=(k == 0), stop=(k == 1),
                    )

            def th_piece(lo, w, ev="v", pool_tag="pw"):
                # theta' = A*(proj + thb): evict applies scale A + A*thb
                pool = pf if pool_tag == "pf" else pw
                pp = pool.tile([128, 512], F32, tag=pool_tag,
                               name=f"thp_{lo}")
                proj_mm(pp, thw, lo, w)
                if ev == "s":
                    nc.scalar.activation(th_sb[:, lo:lo + w], pp[:, 0:w],
                                         AF.Identity, bias=thbA,
                                         scale=float(A_SCALE))
                else:
                    nc.vector.tensor_scalar(
                        th_sb[:, lo:lo + w], pp[:, 0:w],
                        float(A_SCALE), thbA, OP.mult, OP.add)

            def ph_piece(lo, w, ev="v", pool_tag="pw"):
                pool = pf if pool_tag == "pf" else pw
                pp = pool.tile([128, 512], F32, tag=pool_tag,
                               name=f"php_{lo}")
                proj_mm(pp, phw, lo, w)
                if ev == "s":
                    nc.scalar.activation(ph_sb[:, lo:lo + w], pp[:, 0:w],
                                         AF.Identity, bias=phb)
                else:
                    nc.vector.tensor_scalar_add(ph_sb[:, lo:lo + w],
                                                pp[:, 0:w], phb)

            gtiles = {}

            def g_piece(c, half):
                # gT blocks 4c+2*half, 4c+2*half+1 into shared [128,512] tile
                if half == 0:
                    gtiles[c] = pw.tile([128, 512], F32, tag="pw",
                                        name=f"gp_{c}")
                pg = gtiles[c]
                for b in (4 * c + 2 * half, 4 * c + 2 * half + 1):
                    col = (b - 4 * c) * 128
                    for k in range(2):
                        nc.tensor.matmul(
                            pg[:, col:col + 128],
                            xs[k][:, b * 128:(b + 1) * 128],
                            gw[:, k * CI:(k + 1) * CI],
                            start=(k == 0), stop=(k == 1),
                        )
                if half == 1:
                    lo = c * 512
                    nc.vector.tensor_copy(gT_sb[:, lo:lo + 512], pg[:])
                    del gtiles[c]

            def emit_sched_op(op):
                kind = op[0]
                if kind == "th":
                    th_piece(op[1] * 512, 512)
                elif kind == "ph":
                    ph_piece(op[1] * 512, 512)
                elif kind == "ga":
                    g_piece(op[1], 0)
                elif kind == "gb":
                    g_piece(op[1], 1)

            # ---------------- per-quarter deferred ops ----------------
            state = {}   # per-quarter: zq, pzt[2], zi, zbt[2], pyt, ynt
            efs = {}

            def zclose(q, s):
                st = state[q]
                pzt = pw.tile([1, 512], F32, tag="pw", name=f"pz_{q}_{s}")
                st["pzt"][s] = pzt
                nc.tensor.matmul(pzt[:], ones_bf[:],
                                 st["zq"][:, s * 512:(s + 1) * 512],
                                 start=True, stop=True)

            def zinv(q, s):
                # fast variant: ~18 correct bits; Z is far from the edges.
                # zi is F32R so the bcast matmul can consume it at 1 cyc/row
                # (the BIR verifier requires f32r matmul inputs to be
                # *written* as f32r); _custom_dve directly since the
                # wrapper asserts fp32 out.
                from concourse.dve_ops import (
                    RECIP_APPROX_FAST_CONSTS,
                    RECIPROCAL_APPROX_FAST,
                )
                st = state[q]
                if s == 0:
                    st["zi"] = zpool.tile([1, QW], F32R, tag="zi",
                                          name=f"zi_{q}")
                c = RECIP_APPROX_FAST_CONSTS
                nc.vector._custom_dve(
                    RECIPROCAL_APPROX_FAST,
                    out=st["zi"][:, s * 512:(s + 1) * 512],
                    in0=st["pzt"][s][:],
                    s0=c["s0"], s1=c["s1"], imm2=c["imm2"])

            def bcast(q):
                # gpsimd partition broadcast (DVE tensor_tensor cannot read
                # two PSUM operands, so zb must live in SBUF; PE can only
                # write PSUM).  One full-width op: per-half pairs cost an
                # extra ~1.3us gpsimd DRAIN between them.
                st = state[q]
                st["zb"] = zpool.tile([128, QW], F32, tag="zb",
                                      name=f"zb_{q}")
                nc.gpsimd.partition_broadcast(
                    st["zb"][:], st["zi"][:].bitcast(F32))

            def mult(q, s):
                st = state[q]
                if s == 0:
                    st["ynt"] = ypool.tile([128, QW], F32R, tag="ynt",
                                           name=f"ynt_{q}")
                nc.vector.tensor_mul(
                    st["ynt"][:, s * 512:(s + 1) * 512],
                    st["pyt"][:, s * 512:(s + 1) * 512],
                    st["zb"][:, s * 512:(s + 1) * 512])

            def wproj(q, chunk, dma_eng=None):
                ob, s2 = divmod(chunk, 2)
                lo = q * QW + s2 * 512
                pwt = pw.tile([128, 512], F32, tag="pw",
                              name=f"pw_{q}_{chunk}")
                nc.tensor.matmul(
                    pwt[:], ww[:, ob * CI:(ob + 1) * CI],
                    state[q]["ynt"][:, s2 * 512:(s2 + 1) * 512],
                    start=True, stop=True)
                ot = opool.tile([128, 512], F32, tag="o", name=f"o_{q}_{chunk}")
                # out = (W@y/Z + wbe) + x  in one DVE op
                nc.vector.scalar_tensor_tensor(
                    ot[:], pwt[:], wbe[ob], xs[ob][:, lo:lo + 512],
                    OP.add, OP.add)
                (dma_eng or nc.sync).dma_start(
                    out_d[ob * 128:(ob + 1) * 128, lo:lo + 512], ot[:])

            # Z accumulation on DVE (bf16 2x mode): binary-counter tree
            # for j < ZRUN_J, then in-place running adds.
            def tree_push(q, lvl, t_node):
                st = state[q]
                pend = st["pend"]
                if pend.get(lvl) is None:
                    pend[lvl] = t_node
                    return
                a, b = pend.pop(lvl), t_node
                out = ztpool.tile([128, QW], BF16, tag=f"l{lvl}",
                                  name=f"l{lvl}_{q}")
                nc.vector.tensor_add(out[:], a[:], b[:])
                tree_push(q, lvl + 1, out)

            def z_accum(q, j, ef):
                st = state[q]
                if j < ZRUN_J:
                    if j % 2 == 1:
                        pair = ztpool.tile([128, QW], BF16, tag="l0",
                                           name=f"l0_{q}_{j}")
                        nc.vector.tensor_add(pair[:], efs[q * JB + j - 1][:],
                                             ef[:])
                        tree_push(q, 1, pair)
                elif j == ZRUN_J:
                    # merge pending counter partials (sum of 0..23), then run
                    zq = st["zq"] = ztpool.tile([128, QW], BF16, tag="zq",
                                                name=f"zq_{q}")
                    p4, p3 = st["pend"].pop(4), st["pend"].pop(3)
                    nc.vector.tensor_add(zq[:], p4[:], p3[:])
                    nc.vector.tensor_add(zq[:], zq[:], ef[:])
                elif q < NQ - 1 or j < JB - 4:
                    nc.vector.tensor_add(st["zq"][:], st["zq"][:], ef[:])
                else:
                    # last 4 adds of the final quarter in halves: s0 of zq
                    # completes right after the last exp so the tail's
                    # Z-close (subtile dep) starts ~0.5us earlier
                    for s in range(2):
                        sl = slice(s * 512, (s + 1) * 512)
                        nc.vector.tensor_add(st["zq"][:, sl], st["zq"][:, sl],
                                             ef[:, sl])

            def emit_dve_tile(t, ef):
                # DVE fast-exp tiles get their own pw-pool PSUM halves so
                # the slow (1.2us) DVE read never stalls the pf rotation
                # that feeds the Scalar exp pipe (measured ~2us bubbles).
                q, j = divmod(t, JB)
                i0 = q * QW
                for s in range(2):
                    pp = pw.tile([128, 512], F32, tag="pw",
                                 name=f"fd_{t}_{s}")
                    nc.tensor.matmul(
                        pp[:], ph_sb[:, j * 128:(j + 1) * 128],
                        th_sb[:, i0 + s * 512:i0 + (s + 1) * 512],
                        start=True, stop=True)
                    nc.vector.tensor_scalar(
                        ef[:, s * 512:(s + 1) * 512].bitcast(I16), pp[:],
                        float(CB), 0.0, OP.add, OP.max)

            def emit_f(t, pft, s):
                q, j = divmod(t, JB)
                i0 = q * QW
                nc.tensor.matmul(
                    pft[:, s * 512:(s + 1) * 512],
                    ph_sb[:, j * 128:(j + 1) * 128],
                    th_sb[:, i0 + s * 512:i0 + (s + 1) * 512],
                    start=True, stop=True)

            def emit_y(ty):
                qy, jy = divmod(ty, JB)
                if jy == 0:
                    state[qy]["pyt"] = py.tile([128, QW], F32, tag="py",
                                               name=f"py_{qy}")
                efy = efs.pop(ty)
                for s in range(2):
                    nc.tensor.matmul(
                        state[qy]["pyt"][:, s * 512:(s + 1) * 512],
                        gT_sb[:, jy * 128:(jy + 1) * 128],
                        efy[:, s * 512:(s + 1) * 512],
                        start=(jy == 0), stop=(jy == JB - 1))

            # ---------------- pre-loop: th0 / ph0 / th1, split t=0,1 ------
            # PE-queue order matters (in-order engine): f0s0 is emitted as
            # soon as its gates (th0a/th0b/ph0a) are, so exp0a isn't stuck
            # behind ph0b/th1 matmuls.
            state[0] = {"pzt": [None, None], "zbt": [None, None], "pend": {}}
            pf0 = pf.tile([128, QW], F32, tag="pf", name="pf_0")
            pf1 = pf.tile([128, QW], F32, tag="pf", name="pf_1")
            ef0 = efpool.tile([128, QW], BF16, tag="ef", name="ef_0")
            ef1 = efpool.tile([128, QW], BF16, tag="ef", name="ef_1")
            efs[0], efs[1] = ef0, ef1

            th_piece(0, 256, ev="s", pool_tag="pw")
            th_piece(256, 256, ev="v", pool_tag="pw")
            ph_piece(0, 128, ev="v", pool_tag="pw")
            emit_f(0, pf0, 0)
            nc.scalar.activation(ef0[:, 0:512], pf0[:, 0:512], AF.Exp,
                                 bias=aux[:, 5:6], scale=float(SCL))
            ph_piece(128, 384, ev="v", pool_tag="pw")
            emit_f(1, pf1, 0)
            nc.scalar.activation(ef1[:, 0:512], pf1[:, 0:512], AF.Exp,
                                 bias=aux[:, 5:6], scale=float(SCL))
            # s1 halves gated on th chunk 1 (x cols 512:1024)
            th_piece(512, 512, ev="v", pool_tag="pw")
            emit_f(0, pf0, 1)
            emit_f(1, pf1, 1)
            nc.scalar.activation(ef0[:, 512:1024], pf0[:, 512:1024], AF.Exp,
                                 bias=aux[:, 5:6], scale=float(SCL))
            nc.scalar.activation(ef1[:, 512:1024], pf1[:, 512:1024], AF.Exp,
                                 bias=aux[:, 5:6], scale=float(SCL))

            # deferred DMA issues: each issue instruction occupies the
            # queue engine (and blocks on queue-slot availability), so
            # only the pre-loop-critical transfers go ahead of the
            # evictions/exps above.
            nc.sync.dma_start(gw[:, 0:CI], gw_d[0:128, :])
            nc.scalar.dma_start(gw[:, CI:2 * CI], gw_d[128:256, :])
            nc.sync.dma_start(x0[:, 1024:2048], x_d[0:128, 1024:2048])
            nc.scalar.dma_start(x1[:, 1024:2048], x_d[128:256, 1024:2048])
            nc.sync.dma_start(ww[:], ww_d[:])
            nc.sync.dma_start(x0[:, 2048:4096], x_d[0:128, 2048:4096])
            nc.scalar.dma_start(x1[:, 2048:4096], x_d[128:256, 2048:4096])

            # sched ops of t<2 (the main loop starts at t=2): ph chunk 1
            for op in sched.get(0, []) + sched.get(1, []):
                emit_sched_op(op)

            # ---------------- main flat pipeline ----------------
            y_ptr = 0
            for t in range(2, T):
                q, j = divmod(t, JB)
                if j == 0:
                    state[q] = {"pzt": [None, None], "zbt": [None, None],
                                "pend": {}}
                st = state[q]
                ef = efpool.tile([128, QW], BF16, tag="ef", name=f"ef_{t}")
                efs[t] = ef
                if (q, j) in DVE_SET:
                    emit_dve_tile(t, ef)
                else:
                    pft = pf.tile([128, QW], F32, tag="pf", name=f"pf_{t}")
                    emit_f(t, pft, 0)
                    emit_f(t, pft, 1)
                    nc.scalar.activation(ef[:], pft[:], AF.Exp,
                                         bias=aux[:, 5:6], scale=float(SCL))
                if t == 3:
                    z_accum(0, 1, efs[1])   # deferred j=1 pair (split exps)
                z_accum(q, j, ef)
                # previous quarter's deferred work
                if q > 0:
                    if j == ZCLOSE_J[0]:
                        zclose(q - 1, 0)
                    elif j == ZCLOSE_J[1]:
                        zclose(q - 1, 1)
                    elif j == ZINV_J:
                        zinv(q - 1, 0)
                        zinv(q - 1, 1)
                    elif j == BCAST_J[0]:
                        bcast(q - 1)
                    elif j == MULT_J[0]:
                        mult(q - 1, 0)
                    elif j == MULT_J[1]:
                        mult(q - 1, 1)
                    elif WPROJ_J0 <= j < WPROJ_J0 + 4:
                        wproj(q - 1, (0, 2, 1, 3)[j - WPROJ_J0])
                for op in sched.get(t, []):
                    emit_sched_op(op)
                # trailing y accumulation (with q3 lag rampdown)
                if t - YLAG >= 0 and y_ptr <= t - YLAG:
                    emit_y(y_ptr)
                    y_ptr += 1
                if q == NQ - 1 and j in YEXTRA and y_ptr <= t - 2:
                    emit_y(y_ptr)
                    y_ptr += 1

            # ---------------- last quarter's tail (pipelined by half) ------
            q = NQ - 1
            pending = list(range(y_ptr, T))

            def y_half(ty, s):
                qy, jy = divmod(ty, JB)
                nc.tensor.matmul(
                    state[qy]["pyt"][:, s * 512:(s + 1) * 512],
                    gT_sb[:, jy * 128:(jy + 1) * 128],
                    efs[ty][:, s * 512:(s + 1) * 512],
                    start=(jy == 0), stop=(jy == JB - 1))

            # s0 y-drain, then the Z chains for both halves as early as
            # their deps allow (zq-s halves close right after the last
            # exp), with the s1 y-drain interleaved behind zclose(s1).
            for ty in pending:
                y_half(ty, 0)
            zclose(q, 0)
            y_half(pending[0], 1)
            y_half(pending[1], 1)
            zclose(q, 1)
            zinv(q, 0)
            zinv(q, 1)
            bcast(q)
            for ty in pending[2:]:
                y_half(ty, 1)
            mult(q, 0)
            wproj(q, 0, nc.scalar)
            wproj(q, 2)
            mult(q, 1)
            wproj(q, 1, nc.scalar)
            wproj(q, 3)

    nc.compile()
    return nc


_CACHE = {}


def _get_nc():
    if "nc" not in _CACHE:
        _CACHE["nc"] = build()
    return _CACHE["nc"]


def _in_maps(x, g_w, g_b, theta_w, theta_b, phi_w, phi_b, W_w, W_b):
    x = np.ascontiguousarray(np.asarray(x, dtype=np.float32))
    wbe = (np.asarray(W_w, np.float32) @ np.asarray(g_b, np.float32)
           + np.asarray(W_b, np.float32))
    common = {
        "thw_t": np.ascontiguousarray(np.asarray(theta_w, np.float32).T),
        "phw_t": np.ascontiguousarray(np.asarray(phi_w, np.float32).T),
        "gw_t": np.ascontiguousarray(np.asarray(g_w, np.float32).T),
        "ww_t": np.ascontiguousarray(np.asarray(W_w, np.float32).T),
        "aux": np.stack(
            [
                np.asarray(theta_b, np.float32) * np.float32(A_SCALE),
                np.asarray(phi_b, np.float32),
                wbe[:128],
                wbe[128:],
                np.ones(128, np.float32),
                np.full(128, EXP_BIAS, np.float32),
            ],
            axis=1,
        ),
    }
    return [
        {"x": np.ascontiguousarray(x[b].reshape(C, N)), **common}
        for b in range(B)
    ]


def run(in_maps, **kw):
    nc = _get_nc()
    return run_bass_kernel_spmd(nc, in_maps, list(range(B)), **kw)


def kernel(**inputs):
    res = run(_in_maps(**inputs))
    out = np.stack([res.results[b]["out"] for b in range(B)])
    return out.reshape(B, C, H, Wd)


# revision 18
# speedup vs baseline: 1.0510x; 1.0059x over previous
"""NonLocalBlock (embedded-gaussian attention) TRN2 kernel, v3.

Shapes (hardcoded): x [8, 256, 64, 64] fp32, one batch element per core.
Per core:
  theta' = A * (theta_w^T x + theta_b)   (A = 128*log2(e); f' = A*f)
  phi/g  = 1x1 conv projections, [128, 4096]
  f'^T[j, i] = sum_c phi[c, j] theta'[c, i]   (A-scaled 4096x4096 logits)
  Scalar tiles: ef = Exp(f' * (1/A) + (-10))   (global bias -10)
  DVE tiles (9 of 128): Schraudolph bf16 fast-exp in ONE tensor_scalar:
      bits_i16 = trunc(max(f' + CB, 0)), viewed as bf16 ~= e^(f-10)
      (CB = 16251 - A*10; piecewise-linear 2^frac, +-3.3% per elem,
       validated 2.3e-3 rel err end-to-end vs 2e-2 gate)
  Z[i] = sum_j ef[j, i]  via bf16 DVE adds (binary tree then running),
         closed by ones-matmul partition reduce; 1/Z via fast reciprocal;
         Z broadcast via PE ones-row matmul into PSUM (not gpsimd: its
         tensor ops share the DVE SBUF port and inflate DVE ops ~2x)
  y[ci, i] = sum_j ef[j, i] gT[j, ci]   (bf16 matmuls, fp32 PSUM)
  out = x + W_w @ (y * (1/Z)) + wbe,  wbe = W_w @ g_b + W_b folded into
        the W-proj eviction via scalar_tensor_tensor (kills the 16
        bias-add DVE ops of v2)

Engine budget per t (cadence target ~1.03us): Scalar 119 exps x 1.08;
DVE Z-add 0.67 + evictions/mult/STT + 9 fast-exps x 1.24; PE f 0.43 +
y 0.43 + proj/wproj/zclose/bcast ~0.12.

Ramp: 3 DMA queues (sync/scalar hw + gpsimd swdge for weights), first x
pieces 256-col, th0 in 2x256 pieces, t=0/1 exps split in halves so the
Scalar pipe starts ~13.2us instead of 21us.

Tail: y-lag ramps 11->5 over Q3 (extra y slots), then half-pipelined
zclose->zinv->bcastPE->mult->wproj chains alternating DMA queues.
"""

import numpy as np

import concourse.bacc as bacc
import concourse.mybir as mybir
from concourse import tile
from concourse.bass_utils import run_bass_kernel_spmd

F32 = mybir.dt.float32
F32R = mybir.dt.float32r
BF16 = mybir.dt.bfloat16
I16 = mybir.dt.int16
AF = mybir.ActivationFunctionType
OP = mybir.AluOpType

B, C, CI = 8, 256, 128
H, Wd = 64, 64
N = H * Wd              # 4096
NQ = 4                  # i-quarters
QW = N // NQ            # 1024
JB = N // 128           # 32 j-blocks (= ts per quarter)
T = NQ * JB             # 128

YLAG = 11               # steady-state y-matmul lag behind f/exp
YEXTRA = {17, 19, 21, 23, 25, 27, 29, 31}   # q3 rampdown (lag 11->3)
ZCLOSE_J = (3, 4)       # prev quarter's Z partition-reduce MMs
ZINV_J = 5              # prev quarter's reciprocals
BCAST_J = (9, 10)       # prev quarter's Z broadcast (PE ones-row MM)
MULT_J = (11, 12)       # prev quarter's normalize halves
WPROJ_J0 = 13           # prev quarter's W-projection chunks j=13..16
ZRUN_J = 24             # switch Z accumulation from tree to running adds

EXP_BIAS = -10.0        # global logit shift (f range [-90.8, 84.8])
A_SCALE = 128.0 * 1.4426950408889634   # 184.66496...
CB = np.float32(16251.0 + A_SCALE * EXP_BIAS)  # schraudolph add const
SCL = np.float32(1.0 / A_SCALE)
NWARM = 6

# (q, j) tiles whose exp runs on DVE (fast-exp); chosen away from
# deferred-op j slots and sched-heavy ts.  k=9.
DVE_SET = {(1, 6), (1, 20), (1, 26),
           (2, 2), (2, 6), (2, 20), (2, 26),
           (3, 2), (3, 8), (3, 18), (3, 22)}


def _build_sched():
    # t -> list of ops. pieces are 512-col units c=0..7 of x columns.
    # deadlines: ph piece c before f uses j-block 4c (t=4c); th piece c
    # before f of quarter c//2 (t=32*(c//2)); g piece c before y uses
    # block 4c (t=YLAG+4c).  j in {9..16} of q>=1 avoided (pw pool is
    # busy with zb/wproj there).
    sched = {}

    def add(t, op):
        sched.setdefault(t, []).append(op)

    for c in range(1, 8):
        add(4 * c - 3, ("ph", c))
    for c, t in ((2, 24), (3, 26), (4, 50), (5, 54), (6, 82), (7, 86)):
        add(t, ("th", c))
    gsched = {0: 2, 1: 6, 2: 10, 3: 14, 4: 18, 5: 22, 6: 30, 7: 33}
    for c, t in gsched.items():
        add(t, ("ga", c))
        add(t + 1, ("gb", c))
    return sched


def build():
    nc = bacc.Bacc("TRN2", target_bir_lowering=False, debug=False, num_devices=8)

    x_d = nc.dram_tensor("x", [C, N], F32R, kind="ExternalInput")
    thw_d = nc.dram_tensor("thw_t", [C, CI], F32R, kind="ExternalInput")  # theta_w.T
    phw_d = nc.dram_tensor("phw_t", [C, CI], F32R, kind="ExternalInput")  # phi_w.T
    gw_d = nc.dram_tensor("gw_t", [C, CI], F32R, kind="ExternalInput")    # g_w.T
    ww_d = nc.dram_tensor("ww_t", [CI, C], F32R, kind="ExternalInput")    # W_w.T
    # aux cols: 0=A*theta_b, 1=phi_b, 2=wbe[:128], 3=wbe[128:], 4=ones,
    # 5=exp bias (-10)
    aux_d = nc.dram_tensor("aux", [128, 6], F32, kind="ExternalInput")
    out_d = nc.dram_tensor("out", [C, N], F32, kind="ExternalOutput")

    sched = _build_sched()

    with tile.TileContext(nc) as tc:
        with (
            tc.tile_pool(name="const", bufs=1) as cpool,
            tc.tile_pool(name="big", bufs=1) as bigpool,
            tc.tile_pool(name="ef", bufs=16) as efpool,
            tc.tile_pool(name="ztree", bufs=2) as ztpool,
            tc.tile_pool(name="zpool", bufs=2) as zpool,
            tc.tile_pool(name="ypool", bufs=2) as ypool,
            tc.tile_pool(name="opool", bufs=6) as opool,
            tc.tile_pool(name="pf", bufs=2, space="PSUM") as pf,
            tc.tile_pool(name="py", bufs=1, space="PSUM") as py,
            tc.tile_pool(name="pw", bufs=2, space="PSUM") as pw,
        ):
            # ---------------- warmup + DMA issue ----------------
            warm = cpool.tile([128, 512], BF16, tag="warm")
            warm2 = cpool.tile([128, 1], F32, tag="warm2")
            nc.gpsimd.memset(warm[:], 0.0)

            aux = cpool.tile([128, 6], F32, tag="aux")
            thw = cpool.tile([128, 2 * CI], F32R, tag="thw")
            phw = cpool.tile([128, 2 * CI], F32R, tag="phw")
            gw = cpool.tile([128, 2 * CI], F32R, tag="gw")
            ww = cpool.tile([CI, C], F32R, tag="ww")
            x0 = bigpool.tile([128, N], F32R, tag="x0")
            x1 = bigpool.tile([128, N], F32R, tag="x1")
            xs = (x0, x1)

            # x on the two hw queues (256-col first pieces: the rings
            # ramp slowly and the first piece gates the first proj);
            # all weights on the gpsimd software-DGE queue.
            nc.sync.dma_start(aux[:], aux_d[:])
            nc.sync.dma_start(x0[:, 0:256], x_d[0:128, 0:256])
            nc.scalar.dma_start(x1[:, 0:256], x_d[128:256, 0:256])
            nc.sync.dma_start(thw[:, 0:CI], thw_d[0:128, :])
            nc.scalar.dma_start(thw[:, CI:2 * CI], thw_d[128:256, :])
            # dummy activation pulls the ~1.3us exp table load early
            nc.scalar.activation(warm2[:], warm[:, 0:1], AF.Identity)
            nc.sync.dma_start(x0[:, 256:512], x_d[0:128, 256:512])
            nc.scalar.dma_start(x1[:, 256:512], x_d[128:256, 256:512])
            nc.sync.dma_start(phw[:, 0:CI], phw_d[0:128, :])
            nc.scalar.dma_start(phw[:, CI:2 * CI], phw_d[128:256, :])
            nc.sync.dma_start(x0[:, 512:1024], x_d[0:128, 512:1024])
            nc.scalar.dma_start(x1[:, 512:1024], x_d[128:256, 512:1024])

            for _ in range(NWARM):
                pwt = pw.tile([128, 512], F32, tag="pw", name="warm_mm")
                nc.tensor.matmul(pwt[:], warm[:, 0:128], warm[:],
                                 start=True, stop=True)

            thbA = aux[:, 0:1]   # A * theta_b
            phb = aux[:, 1:2]
            wbe = (aux[:, 2:3], aux[:, 3:4])
            ones_bf = cpool.tile([128, 1], BF16, tag="ones_bf")
            nc.vector.tensor_copy(ones_bf[:], aux[:, 4:5])

            th_sb = bigpool.tile([128, N], F32R, tag="th")
            ph_sb = bigpool.tile([128, N], F32R, tag="ph")
            gT_sb = bigpool.tile([128, N], BF16, tag="gT")

            # ---------------- projection piece emitters ----------------
            def proj_mm(pp, wt, lo, w):
                for k in range(2):
                    nc.tensor.matmul(
                        pp[:, 0:w], wt[:, k * CI:(k + 1) * CI],
                        xs[k][:, lo:lo + w],
                        start=(k == 0), stop=(k == 1),
                    )

            def th_piece(lo, w, ev="v", pool_tag="pw"):
                # theta' = A*(proj + thb): evict applies scale A + A*thb
                pool = pf if pool_tag == "pf" else pw
                pp = pool.tile([128, 512], F32, tag=pool_tag,
                               name=f"thp_{lo}")
                proj_mm(pp, thw, lo, w)
                if ev == "s":
                    nc.scalar.activation(th_sb[:, lo:lo + w], pp[:, 0:w],
                                         AF.Identity, bias=thbA,
                                         scale=float(A_SCALE))
                else:
                    nc.vector.tensor_scalar(
                        th_sb[:, lo:lo + w], pp[:, 0:w],
                        float(A_SCALE), thbA, OP.mult, OP.add)

            def ph_piece(lo, w, ev="v", pool_tag="pw"):
                pool = pf if pool_tag == "pf" else pw
                pp = pool.tile([128, 512], F32, tag=pool_tag,
                               name=f"php_{lo}")
                proj_mm(pp, phw, lo, w)
                if ev == "s":
                    nc.scalar.activation(ph_sb[:, lo:lo + w], pp[:, 0:w],
                                         AF.Identity, bias=phb)
                else:
                    nc.vector.tensor_scalar_add(ph_sb[:, lo:lo + w],
                                                pp[:, 0:w], phb)

            gtiles = {}

            def g_piece(c, half):
                # gT blocks 4c+2*half, 4c+2*half+1 into shared [128,512] tile
                if half == 0:
                    gtiles[c] = pw.tile([128, 512], F32, tag="pw",
                                        name=f"gp_{c}")
                pg = gtiles[c]
                for b in (4 * c + 2 * half, 4 * c + 2 * half + 1):
                    col = (b - 4 * c) * 128
                    for k in range(2):
                        nc.tensor.matmul(
                            pg[:, col:col + 128],
                            xs[k][:, b * 128:(b + 1) * 128],
                            gw[:, k * CI:(k + 1) * CI],
                            start=(k == 0), stop=(k == 1),
                        )
                if half == 1:
                    lo = c * 512
                    nc.vector.tensor_copy(gT_sb[:, lo:lo + 512], pg[:])
                    del gtiles[c]

            def emit_sched_op(op):
                kind = op[0]
                if kind == "th":
                    th_piece(op[1] * 512, 512)
                elif kind == "ph":
                    ph_piece(op[1] * 512, 512)
                elif kind == "ga":
                    g_piece(op[1], 0)
                elif kind == "gb":
                    g_piece(op[1], 1)

            # ---------------- per-quarter deferred ops ----------------
            state = {}   # per-quarter: zq, pzt[2], zi, zbt[2], pyt, ynt
            efs = {}

            def zclose(q, s):
                st = state[q]
                pzt = pw.tile([1, 512], F32, tag="pw", name=f"pz_{q}_{s}")
                st["pzt"][s] = pzt
                nc.tensor.matmul(pzt[:], ones_bf[:],
                                 st["zq"][:, s * 512:(s + 1) * 512],
                                 start=True, stop=True)

            def zinv(q, s):
                # fast variant: ~18 correct bits; Z is far from the edges.
                # zi is F32R so the bcast matmul can consume it at 1 cyc/row
                # (the BIR verifier requires f32r matmul inputs to be
                # *written* as f32r); _custom_dve directly since the
                # wrapper asserts fp32 out.
                from concourse.dve_ops import (
                    RECIP_APPROX_FAST_CONSTS,
                    RECIPROCAL_APPROX_FAST,
                )
                st = state[q]
                if s == 0:
                    st["zi"] = zpool.tile([1, QW], F32R, tag="zi",
                                          name=f"zi_{q}")
                c = RECIP_APPROX_FAST_CONSTS
                nc.vector._custom_dve(
                    RECIPROCAL_APPROX_FAST,
                    out=st["zi"][:, s * 512:(s + 1) * 512],
                    in0=st["pzt"][s][:],
                    s0=c["s0"], s1=c["s1"], imm2=c["imm2"])

            def bcast(q, s):
                # gpsimd partition broadcast (DVE tensor_tensor cannot read
                # two PSUM operands, so zb must live in SBUF; PE can only
                # write PSUM).  gpsimd is otherwise idle.
                st = state[q]
                if s == 0:
                    st["zb"] = zpool.tile([128, QW], F32, tag="zb",
                                          name=f"zb_{q}")
                sl = slice(s * 512, (s + 1) * 512)
                nc.gpsimd.partition_broadcast(
                    st["zb"][:, sl], st["zi"][:, sl].bitcast(F32))

            def mult(q, s):
                st = state[q]
                if s == 0:
                    st["ynt"] = ypool.tile([128, QW], F32R, tag="ynt",
                                           name=f"ynt_{q}")
                nc.vector.tensor_mul(
                    st["ynt"][:, s * 512:(s + 1) * 512],
                    st["pyt"][:, s * 512:(s + 1) * 512],
                    st["zb"][:, s * 512:(s + 1) * 512])

            def wproj(q, chunk, dma_eng=None):
                ob, s2 = divmod(chunk, 2)
                lo = q * QW + s2 * 512
                pwt = pw.tile([128, 512], F32, tag="pw",
                              name=f"pw_{q}_{chunk}")
                nc.tensor.matmul(
                    pwt[:], ww[:, ob * CI:(ob + 1) * CI],
                    state[q]["ynt"][:, s2 * 512:(s2 + 1) * 512],
                    start=True, stop=True)
                ot = opool.tile([128, 512], F32, tag="o", name=f"o_{q}_{chunk}")
                # out = (W@y/Z + wbe) + x  in one DVE op
                nc.vector.scalar_tensor_tensor(
                    ot[:], pwt[:], wbe[ob], xs[ob][:, lo:lo + 512],
                    OP.add, OP.add)
                (dma_eng or nc.sync).dma_start(
                    out_d[ob * 128:(ob + 1) * 128, lo:lo + 512], ot[:])

            # Z accumulation on DVE (bf16 2x mode): binary-counter tree
            # for j < ZRUN_J, then in-place running adds.
            def tree_push(q, lvl, t_node):
                st = state[q]
                pend = st["pend"]
                if pend.get(lvl) is None:
                    pend[lvl] = t_node
                    return
                a, b = pend.pop(lvl), t_node
                out = ztpool.tile([128, QW], BF16, tag=f"l{lvl}",
                                  name=f"l{lvl}_{q}")
                nc.vector.tensor_add(out[:], a[:], b[:])
                tree_push(q, lvl + 1, out)

            def z_accum(q, j, ef):
                st = state[q]
                if j < ZRUN_J:
                    if j % 2 == 1:
                        pair = ztpool.tile([128, QW], BF16, tag="l0",
                                           name=f"l0_{q}_{j}")
                        nc.vector.tensor_add(pair[:], efs[q * JB + j - 1][:],
                                             ef[:])
                        tree_push(q, 1, pair)
                elif j == ZRUN_J:
                    # merge pending counter partials (sum of 0..23), then run
                    zq = st["zq"] = ztpool.tile([128, QW], BF16, tag="zq",
                                                name=f"zq_{q}")
                    p4, p3 = st["pend"].pop(4), st["pend"].pop(3)
                    nc.vector.tensor_add(zq[:], p4[:], p3[:])
                    nc.vector.tensor_add(zq[:], zq[:], ef[:])
                elif q < NQ - 1 or j < JB - 4:
                    nc.vector.tensor_add(st["zq"][:], st["zq"][:], ef[:])
                else:
                    # final quarter, j>=28: close the 28-tile partial into
                    # pz (PE ones-MM, start) and accumulate the last tiles'
                    # ef directly -- no DVE work trails the last exp and
                    # the tail needs no zclose.
                    if j == JB - 4:
                        for s in range(2):
                            pzt = pw.tile([1, 512], F32, tag="pw",
                                          name=f"pz3_{s}")
                            st["pzt"][s] = pzt
                            nc.tensor.matmul(
                                pzt[:], ones_bf[:],
                                st["zq"][:, s * 512:(s + 1) * 512],
                                start=True, stop=False)
                    for s in range(2):
                        nc.tensor.matmul(
                            st["pzt"][s][:], ones_bf[:],
                            ef[:, s * 512:(s + 1) * 512],
                            start=False, stop=(j == JB - 1))

            def emit_dve_tile(t, ef):
                # DVE fast-exp tiles get their own pw-pool PSUM halves so
                # the slow (1.2us) DVE read never stalls the pf rotation
                # that feeds the Scalar exp pipe (measured ~2us bubbles).
                q, j = divmod(t, JB)
                i0 = q * QW
                for s in range(2):
                    pp = pw.tile([128, 512], F32, tag="pw",
                                 name=f"fd_{t}_{s}")
                    nc.tensor.matmul(
                        pp[:], ph_sb[:, j * 128:(j + 1) * 128],
                        th_sb[:, i0 + s * 512:i0 + (s + 1) * 512],
                        start=True, stop=True)
                    nc.vector.tensor_scalar(
                        ef[:, s * 512:(s + 1) * 512].bitcast(I16), pp[:],
                        float(CB), 0.0, OP.add, OP.max)

            def emit_f(t, pft, s):
                q, j = divmod(t, JB)
                i0 = q * QW
                nc.tensor.matmul(
                    pft[:, s * 512:(s + 1) * 512],
                    ph_sb[:, j * 128:(j + 1) * 128],
                    th_sb[:, i0 + s * 512:i0 + (s + 1) * 512],
                    start=True, stop=True)

            def emit_y(ty):
                qy, jy = divmod(ty, JB)
                if jy == 0:
                    state[qy]["pyt"] = py.tile([128, QW], F32, tag="py",
                                               name=f"py_{qy}")
                efy = efs.pop(ty)
                for s in range(2):
                    nc.tensor.matmul(
                        state[qy]["pyt"][:, s * 512:(s + 1) * 512],
                        gT_sb[:, jy * 128:(jy + 1) * 128],
                        efy[:, s * 512:(s + 1) * 512],
                        start=(jy == 0), stop=(jy == JB - 1))

            # ---------------- pre-loop: th0 / ph0 / th1, split t=0,1 ------
            # PE-queue order matters (in-order engine): f0s0 is emitted as
            # soon as its gates (th0a/th0b/ph0a) are, so exp0a isn't stuck
            # behind ph0b/th1 matmuls.
            state[0] = {"pzt": [None, None], "zbt": [None, None], "pend": {}}
            pf0 = pf.tile([128, QW], F32, tag="pf", name="pf_0")
            pf1 = pf.tile([128, QW], F32, tag="pf", name="pf_1")
            ef0 = efpool.tile([128, QW], BF16, tag="ef", name="ef_0")
            ef1 = efpool.tile([128, QW], BF16, tag="ef", name="ef_1")
            efs[0], efs[1] = ef0, ef1

            th_piece(0, 256, ev="s", pool_tag="pw")
            th_piece(256, 256, ev="v", pool_tag="pw")
            ph_piece(0, 128, ev="v", pool_tag="pw")
            emit_f(0, pf0, 0)
            nc.scalar.activation(ef0[:, 0:512], pf0[:, 0:512], AF.Exp,
                                 bias=aux[:, 5:6], scale=float(SCL))
            ph_piece(128, 384, ev="v", pool_tag="pw")
            emit_f(1, pf1, 0)
            nc.scalar.activation(ef1[:, 0:512], pf1[:, 0:512], AF.Exp,
                                 bias=aux[:, 5:6], scale=float(SCL))
            # s1 halves gated on th chunk 1 (x cols 512:1024)
            th_piece(512, 512, ev="v", pool_tag="pw")
            emit_f(0, pf0, 1)
            emit_f(1, pf1, 1)
            nc.scalar.activation(ef0[:, 512:1024], pf0[:, 512:1024], AF.Exp,
                                 bias=aux[:, 5:6], scale=float(SCL))
            nc.scalar.activation(ef1[:, 512:1024], pf1[:, 512:1024], AF.Exp,
                                 bias=aux[:, 5:6], scale=float(SCL))

            # deferred DMA issues: each issue instruction occupies the
            # queue engine (and blocks on queue-slot availability), so
            # only the pre-loop-critical transfers go ahead of the
            # evictions/exps above.
            nc.sync.dma_start(x0[:, 1024:2048], x_d[0:128, 1024:2048])
            nc.sync.dma_start(x1[:, 1024:2048], x_d[128:256, 1024:2048])
            nc.sync.dma_start(gw[:, 0:CI], gw_d[0:128, :])
            nc.sync.dma_start(gw[:, CI:2 * CI], gw_d[128:256, :])
            nc.sync.dma_start(ww[:], ww_d[:])
            nc.sync.dma_start(x0[:, 2048:4096], x_d[0:128, 2048:4096])
            nc.sync.dma_start(x1[:, 2048:4096], x_d[128:256, 2048:4096])

            # sched ops of t<2 (the main loop starts at t=2): ph chunk 1
            for op in sched.get(0, []) + sched.get(1, []):
                emit_sched_op(op)

            # ---------------- main flat pipeline ----------------
            y_ptr = 0
            for t in range(2, T):
                q, j = divmod(t, JB)
                if j == 0:
                    state[q] = {"pzt": [None, None], "zbt": [None, None],
                                "pend": {}}
                st = state[q]
                ef = efpool.tile([128, QW], BF16, tag="ef", name=f"ef_{t}")
                efs[t] = ef
                if (q, j) in DVE_SET:
                    emit_dve_tile(t, ef)
                else:
                    pft = pf.tile([128, QW], F32, tag="pf", name=f"pf_{t}")
                    emit_f(t, pft, 0)
                    emit_f(t, pft, 1)
                    nc.scalar.activation(ef[:], pft[:], AF.Exp,
                                         bias=aux[:, 5:6], scale=float(SCL))
                if t == 3:
                    z_accum(0, 1, efs[1])   # deferred j=1 pair (split exps)
                z_accum(q, j, ef)
                # previous quarter's deferred work
                if q > 0:
                    if j == ZCLOSE_J[0]:
                        zclose(q - 1, 0)
                    elif j == ZCLOSE_J[1]:
                        zclose(q - 1, 1)
                    elif j == ZINV_J:
                        zinv(q - 1, 0)
                        zinv(q - 1, 1)
                    elif j == BCAST_J[0]:
                        bcast(q - 1, 0)
                    elif j == BCAST_J[1]:
                        bcast(q - 1, 1)
                    elif j == MULT_J[0]:
                        mult(q - 1, 0)
                    elif j == MULT_J[1]:
                        mult(q - 1, 1)
                    elif WPROJ_J0 <= j < WPROJ_J0 + 4:
                        wproj(q - 1, (0, 2, 1, 3)[j - WPROJ_J0])
                for op in sched.get(t, []):
                    emit_sched_op(op)
                # trailing y accumulation (with q3 lag rampdown)
                if t - YLAG >= 0 and y_ptr <= t - YLAG:
                    emit_y(y_ptr)
                    y_ptr += 1
                if q == NQ - 1 and j in YEXTRA and y_ptr <= t - 2:
                    emit_y(y_ptr)
                    y_ptr += 1

            # ---------------- last quarter's tail (pipelined by half) ------
            q = NQ - 1
            pending = list(range(y_ptr, T))

            def y_half(ty, s):
                qy, jy = divmod(ty, JB)
                nc.tensor.matmul(
                    state[qy]["pyt"][:, s * 512:(s + 1) * 512],
                    gT_sb[:, jy * 128:(jy + 1) * 128],
                    efs[ty][:, s * 512:(s + 1) * 512],
                    start=(jy == 0), stop=(jy == JB - 1))

            # s0 y-drain, then the Z chains for both halves as early as
            # their deps allow (zq-s halves close right after the last
            # exp), with the s1 y-drain interleaved behind zclose(s1).
            for ty in pending:
                y_half(ty, 0)
            zinv(q, 0)
            zinv(q, 1)
            bcast(q, 0)
            for ty in pending:
                y_half(ty, 1)
            bcast(q, 1)
            mult(q, 0)
            wproj(q, 0, nc.scalar)
            wproj(q, 2)
            mult(q, 1)
            wproj(q, 1, nc.scalar)
            wproj(q, 3)

    nc.compile()
    return nc


_CACHE = {}


def _get_nc():
    if "nc" not in _CACHE:
        _CACHE["nc"] = build()
    return _CACHE["nc"]


def _in_maps(x, g_w, g_b, theta_w, theta_b, phi_w, phi_b, W_w, W_b):
    x = np.ascontiguousarray(np.asarray(x, dtype=np.float32))
    wbe = (np.asarray(W_w, np.float32) @ np.asarray(g_b, np.float32)
           + np.asarray(W_b, np.float32))
    common = {
        "thw_t": np.ascontiguousarray(np.asarray(theta_w, np.float32).T),
        "phw_t": np.ascontiguousarray(np.asarray(phi_w, np.float32).T),
        "gw_t": np.ascontiguousarray(np.asarray(g_w, np.float32).T),
        "ww_t": np.ascontiguousarray(np.asarray(W_w, np.float32).T),
        "aux": np.stack(
            [
                np.asarray(theta_b, np.float32) * np.float32(A_SCALE),
                np.asarray(phi_b, np.float32),
                wbe[:128],
                wbe[128:],
                np.ones(128, np.float32),
                np.full(128, EXP_BIAS, np.float32),
            ],
            axis=1,
        ),
    }
    return [
        {"x": np.ascontiguousarray(x[b].reshape(C, N)), **common}
        for b in range(B)
    ]


def run(in_maps, **kw):
    nc = _get_nc()
    return run_bass_kernel_spmd(nc, in_maps, list(range(B)), **kw)


def kernel(**inputs):
    res = run(_in_maps(**inputs))
    out = np.stack([res.results[b]["out"] for b in range(B)])
    return out.reshape(B, C, H, Wd)


# revision 20
# speedup vs baseline: 1.0696x; 1.0176x over previous
"""NonLocalBlock (embedded-gaussian attention) TRN2 kernel, v3.

Shapes (hardcoded): x [8, 256, 64, 64] fp32, one batch element per core.
Per core:
  theta' = A * (theta_w^T x + theta_b)   (A = 128*log2(e); f' = A*f)
  phi/g  = 1x1 conv projections, [128, 4096]
  f'^T[j, i] = sum_c phi[c, j] theta'[c, i]   (A-scaled 4096x4096 logits)
  Scalar tiles: ef = Exp(f' * (1/A) + (-10))   (global bias -10)
  DVE tiles (9 of 128): Schraudolph bf16 fast-exp in ONE tensor_scalar:
      bits_i16 = trunc(max(f' + CB, 0)), viewed as bf16 ~= e^(f-10)
      (CB = 16251 - A*10; piecewise-linear 2^frac, +-3.3% per elem,
       validated 2.3e-3 rel err end-to-end vs 2e-2 gate)
  Z[i] = sum_j ef[j, i]  via bf16 DVE adds (binary tree then running),
         closed by ones-matmul partition reduce; 1/Z via fast reciprocal;
         Z broadcast via PE ones-row matmul into PSUM (not gpsimd: its
         tensor ops share the DVE SBUF port and inflate DVE ops ~2x)
  y[ci, i] = sum_j ef[j, i] gT[j, ci]   (bf16 matmuls, fp32 PSUM)
  out = x + W_w @ (y * (1/Z)) + wbe,  wbe = W_w @ g_b + W_b folded into
        the W-proj eviction via scalar_tensor_tensor (kills the 16
        bias-add DVE ops of v2)

Engine budget per t (cadence target ~1.03us): Scalar 119 exps x 1.08;
DVE Z-add 0.67 + evictions/mult/STT + 9 fast-exps x 1.24; PE f 0.43 +
y 0.43 + proj/wproj/zclose/bcast ~0.12.

Ramp: 3 DMA queues (sync/scalar hw + gpsimd swdge for weights), first x
pieces 256-col, th0 in 2x256 pieces, t=0/1 exps split in halves so the
Scalar pipe starts ~13.2us instead of 21us.

Tail: y-lag ramps 11->5 over Q3 (extra y slots), then half-pipelined
zclose->zinv->bcastPE->mult->wproj chains alternating DMA queues.
"""

import numpy as np

import concourse.bacc as bacc
import concourse.mybir as mybir
from concourse import tile
from concourse.bass_utils import run_bass_kernel_spmd

F32 = mybir.dt.float32
F32R = mybir.dt.float32r
BF16 = mybir.dt.bfloat16
I16 = mybir.dt.int16
AF = mybir.ActivationFunctionType
OP = mybir.AluOpType

B, C, CI = 8, 256, 128
H, Wd = 64, 64
N = H * Wd              # 4096
NQ = 4                  # i-quarters
QW = N // NQ            # 1024
JB = N // 128           # 32 j-blocks (= ts per quarter)
T = NQ * JB             # 128

YLAG = 11               # steady-state y-matmul lag behind f/exp
YEXTRA = {17, 19, 21, 23, 25, 27}   # q3 slots emitting a 2nd y (lag 11->5)
ZCLOSE_J = (3, 4)       # prev quarter's Z partition-reduce MMs
ZINV_J = 5              # prev quarter's reciprocals
BCAST_J = (9, 10)       # prev quarter's Z broadcast (PE ones-row MM)
MULT_J = (11, 12)       # prev quarter's normalize halves
WPROJ_J0 = 13           # prev quarter's W-projection chunks j=13..16
ZRUN_J = 24             # switch Z accumulation from tree to running adds

EXP_BIAS = -10.0        # global logit shift (f range [-90.8, 84.8])
A_SCALE = 128.0 * 1.4426950408889634   # 184.66496...
CB = np.float32(16251.0 + A_SCALE * EXP_BIAS)  # schraudolph add const
SCL = np.float32(1.0 / A_SCALE)
NWARM = 6

# (q, j) tiles whose exp runs on DVE (fast-exp); chosen away from
# deferred-op j slots and sched-heavy ts.  k=9.
DVE_SET = {(1, 6), (1, 20), (1, 26),
           (2, 2), (2, 6), (2, 20), (2, 26),
           (3, 2), (3, 8), (3, 18), (3, 22)}


def _build_sched():
    # t -> list of ops. pieces are 512-col units c=0..7 of x columns.
    # deadlines: ph piece c before f uses j-block 4c (t=4c); th piece c
    # before f of quarter c//2 (t=32*(c//2)); g piece c before y uses
    # block 4c (t=YLAG+4c).  j in {9..16} of q>=1 avoided (pw pool is
    # busy with zb/wproj there).
    sched = {}

    def add(t, op):
        sched.setdefault(t, []).append(op)

    for c in range(1, 8):
        add(4 * c - 3, ("ph", c))
    for c, t in ((2, 24), (3, 26), (4, 50), (5, 54), (6, 82), (7, 86)):
        add(t, ("th", c))
    gsched = {0: 2, 1: 6, 2: 10, 3: 14, 4: 18, 5: 22, 6: 30, 7: 33}
    for c, t in gsched.items():
        add(t, ("ga", c))
        add(t + 1, ("gb", c))
    return sched


def build():
    nc = bacc.Bacc("TRN2", target_bir_lowering=False, debug=False, num_devices=8)

    x_d = nc.dram_tensor("x", [C, N], F32R, kind="ExternalInput")
    thw_d = nc.dram_tensor("thw_t", [C, CI], F32R, kind="ExternalInput")  # theta_w.T
    phw_d = nc.dram_tensor("phw_t", [C, CI], F32R, kind="ExternalInput")  # phi_w.T
    gw_d = nc.dram_tensor("gw_t", [C, CI], F32R, kind="ExternalInput")    # g_w.T
    ww_d = nc.dram_tensor("ww_t", [CI, C], F32R, kind="ExternalInput")    # W_w.T
    # aux cols: 0=A*theta_b, 1=phi_b, 2=wbe[:128], 3=wbe[128:], 4=ones,
    # 5=exp bias (-10)
    aux_d = nc.dram_tensor("aux", [128, 6], F32, kind="ExternalInput")
    out_d = nc.dram_tensor("out", [C, N], F32, kind="ExternalOutput")

    sched = _build_sched()

    with tile.TileContext(nc) as tc:
        with (
            tc.tile_pool(name="const", bufs=1) as cpool,
            tc.tile_pool(name="big", bufs=1) as bigpool,
            tc.tile_pool(name="ef", bufs=16) as efpool,
            tc.tile_pool(name="ztree", bufs=2) as ztpool,
            tc.tile_pool(name="zpool", bufs=2) as zpool,
            tc.tile_pool(name="ypool", bufs=2) as ypool,
            tc.tile_pool(name="opool", bufs=6) as opool,
            tc.tile_pool(name="pf", bufs=2, space="PSUM") as pf,
            tc.tile_pool(name="py", bufs=1, space="PSUM") as py,
            tc.tile_pool(name="pw", bufs=2, space="PSUM") as pw,
        ):
            # ---------------- warmup + DMA issue ----------------
            warm = cpool.tile([128, 512], BF16, tag="warm")
            warm2 = cpool.tile([128, 1], F32, tag="warm2")
            nc.gpsimd.memset(warm[:], 0.0)

            aux = cpool.tile([128, 6], F32, tag="aux")
            thw = cpool.tile([128, 2 * CI], F32R, tag="thw")
            phw = cpool.tile([128, 2 * CI], F32R, tag="phw")
            gw = cpool.tile([128, 2 * CI], F32R, tag="gw")
            ww = cpool.tile([CI, C], F32R, tag="ww")
            x0 = bigpool.tile([128, N], F32R, tag="x0")
            x1 = bigpool.tile([128, N], F32R, tag="x1")
            xs = (x0, x1)

            # x on the two hw queues (256-col first pieces: the rings
            # ramp slowly and the first piece gates the first proj);
            # all weights on the gpsimd software-DGE queue.
            nc.sync.dma_start(aux[:], aux_d[:])
            nc.sync.dma_start(x0[:, 0:256], x_d[0:128, 0:256])
            nc.scalar.dma_start(x1[:, 0:256], x_d[128:256, 0:256])
            nc.sync.dma_start(thw[:, 0:CI], thw_d[0:128, :])
            nc.scalar.dma_start(thw[:, CI:2 * CI], thw_d[128:256, :])
            # dummy activation pulls the ~1.3us exp table load early
            nc.scalar.activation(warm2[:], warm[:, 0:1], AF.Identity)
            nc.sync.dma_start(x0[:, 256:512], x_d[0:128, 256:512])
            nc.scalar.dma_start(x1[:, 256:512], x_d[128:256, 256:512])
            nc.sync.dma_start(phw[:, 0:CI], phw_d[0:128, :])
            nc.scalar.dma_start(phw[:, CI:2 * CI], phw_d[128:256, :])
            nc.sync.dma_start(x0[:, 512:1024], x_d[0:128, 512:1024])
            nc.scalar.dma_start(x1[:, 512:1024], x_d[128:256, 512:1024])

            for _ in range(NWARM):
                pwt = pw.tile([128, 512], F32, tag="pw", name="warm_mm")
                nc.tensor.matmul(pwt[:], warm[:, 0:128], warm[:],
                                 start=True, stop=True)

            thbA = aux[:, 0:1]   # A * theta_b
            phb = aux[:, 1:2]
            wbe = (aux[:, 2:3], aux[:, 3:4])
            ones_bf = cpool.tile([128, 1], BF16, tag="ones_bf")
            nc.vector.tensor_copy(ones_bf[:], aux[:, 4:5])

            th_sb = bigpool.tile([128, N], F32R, tag="th")
            ph_sb = bigpool.tile([128, N], F32R, tag="ph")
            gT_sb = bigpool.tile([128, N], BF16, tag="gT")

            # ---------------- projection piece emitters ----------------
            def proj_mm(pp, wt, lo, w):
                for k in range(2):
                    nc.tensor.matmul(
                        pp[:, 0:w], wt[:, k * CI:(k + 1) * CI],
                        xs[k][:, lo:lo + w],
                        start=(k == 0), stop=(k == 1),
                    )

            def th_piece(lo, w, ev="v", pool_tag="pw"):
                # theta' = A*(proj + thb): evict applies scale A + A*thb
                pool = pf if pool_tag == "pf" else pw
                pp = pool.tile([128, 512], F32, tag=pool_tag,
                               name=f"thp_{lo}")
                proj_mm(pp, thw, lo, w)
                if ev == "s":
                    nc.scalar.activation(th_sb[:, lo:lo + w], pp[:, 0:w],
                                         AF.Identity, bias=thbA,
                                         scale=float(A_SCALE))
                else:
                    nc.vector.tensor_scalar(
                        th_sb[:, lo:lo + w], pp[:, 0:w],
                        float(A_SCALE), thbA, OP.mult, OP.add)

            def ph_piece(lo, w, ev="v", pool_tag="pw"):
                pool = pf if pool_tag == "pf" else pw
                pp = pool.tile([128, 512], F32, tag=pool_tag,
                               name=f"php_{lo}")
                proj_mm(pp, phw, lo, w)
                if ev == "s":
                    nc.scalar.activation(ph_sb[:, lo:lo + w], pp[:, 0:w],
                                         AF.Identity, bias=phb)
                else:
                    nc.vector.tensor_scalar_add(ph_sb[:, lo:lo + w],
                                                pp[:, 0:w], phb)

            gtiles = {}

            def g_piece(c, half):
                # gT blocks 4c+2*half, 4c+2*half+1 into shared [128,512] tile
                if half == 0:
                    gtiles[c] = pw.tile([128, 512], F32, tag="pw",
                                        name=f"gp_{c}")
                pg = gtiles[c]
                for b in (4 * c + 2 * half, 4 * c + 2 * half + 1):
                    col = (b - 4 * c) * 128
                    for k in range(2):
                        nc.tensor.matmul(
                            pg[:, col:col + 128],
                            xs[k][:, b * 128:(b + 1) * 128],
                            gw[:, k * CI:(k + 1) * CI],
                            start=(k == 0), stop=(k == 1),
                        )
                if half == 1:
                    lo = c * 512
                    nc.vector.tensor_copy(gT_sb[:, lo:lo + 512], pg[:])
                    del gtiles[c]

            def emit_sched_op(op):
                kind = op[0]
                if kind == "th":
                    th_piece(op[1] * 512, 512)
                elif kind == "ph":
                    ph_piece(op[1] * 512, 512)
                elif kind == "ga":
                    g_piece(op[1], 0)
                elif kind == "gb":
                    g_piece(op[1], 1)

            # ---------------- per-quarter deferred ops ----------------
            state = {}   # per-quarter: zq, pzt[2], zi, zbt[2], pyt, ynt
            efs = {}

            def zclose(q, s):
                st = state[q]
                pzt = pw.tile([1, 512], F32, tag="pw", name=f"pz_{q}_{s}")
                st["pzt"][s] = pzt
                nc.tensor.matmul(pzt[:], ones_bf[:],
                                 st["zq"][:, s * 512:(s + 1) * 512],
                                 start=True, stop=True)

            def zinv(q, s):
                # fast variant: ~18 correct bits; Z is far from the edges.
                # zi is F32R so the bcast matmul can consume it at 1 cyc/row
                # (the BIR verifier requires f32r matmul inputs to be
                # *written* as f32r); _custom_dve directly since the
                # wrapper asserts fp32 out.
                from concourse.dve_ops import (
                    RECIP_APPROX_FAST_CONSTS,
                    RECIPROCAL_APPROX_FAST,
                )
                st = state[q]
                if s == 0:
                    st["zi"] = zpool.tile([1, QW], F32R, tag="zi",
                                          name=f"zi_{q}")
                c = RECIP_APPROX_FAST_CONSTS
                nc.vector._custom_dve(
                    RECIPROCAL_APPROX_FAST,
                    out=st["zi"][:, s * 512:(s + 1) * 512],
                    in0=st["pzt"][s][:],
                    s0=c["s0"], s1=c["s1"], imm2=c["imm2"])

            def bcast(q, s):
                # gpsimd partition broadcast (DVE tensor_tensor cannot read
                # two PSUM operands, so zb must live in SBUF; PE can only
                # write PSUM).  gpsimd is otherwise idle.
                st = state[q]
                if s == 0:
                    st["zb"] = zpool.tile([128, QW], F32, tag="zb",
                                          name=f"zb_{q}")
                sl = slice(s * 512, (s + 1) * 512)
                nc.gpsimd.partition_broadcast(
                    st["zb"][:, sl], st["zi"][:, sl].bitcast(F32))

            def mult(q, s):
                st = state[q]
                if s == 0:
                    st["ynt"] = ypool.tile([128, QW], F32R, tag="ynt",
                                           name=f"ynt_{q}")
                nc.vector.tensor_mul(
                    st["ynt"][:, s * 512:(s + 1) * 512],
                    st["pyt"][:, s * 512:(s + 1) * 512],
                    st["zb"][:, s * 512:(s + 1) * 512])

            def wproj(q, chunk, dma_eng=None):
                ob, s2 = divmod(chunk, 2)
                lo = q * QW + s2 * 512
                pwt = pw.tile([128, 512], F32, tag="pw",
                              name=f"pw_{q}_{chunk}")
                nc.tensor.matmul(
                    pwt[:], ww[:, ob * CI:(ob + 1) * CI],
                    state[q]["ynt"][:, s2 * 512:(s2 + 1) * 512],
                    start=True, stop=True)
                ot = opool.tile([128, 512], F32, tag="o", name=f"o_{q}_{chunk}")
                # out = (W@y/Z + wbe) + x  in one DVE op
                nc.vector.scalar_tensor_tensor(
                    ot[:], pwt[:], wbe[ob], xs[ob][:, lo:lo + 512],
                    OP.add, OP.add)
                (dma_eng or nc.sync).dma_start(
                    out_d[ob * 128:(ob + 1) * 128, lo:lo + 512], ot[:])

            # Z accumulation on DVE (bf16 2x mode): binary-counter tree
            # for j < ZRUN_J, then in-place running adds.
            def tree_push(q, lvl, t_node):
                st = state[q]
                pend = st["pend"]
                if pend.get(lvl) is None:
                    pend[lvl] = t_node
                    return
                a, b = pend.pop(lvl), t_node
                out = ztpool.tile([128, QW], BF16, tag=f"l{lvl}",
                                  name=f"l{lvl}_{q}")
                nc.vector.tensor_add(out[:], a[:], b[:])
                tree_push(q, lvl + 1, out)

            def z_accum(q, j, ef):
                st = state[q]
                if j < ZRUN_J:
                    if j % 2 == 1:
                        pair = ztpool.tile([128, QW], BF16, tag="l0",
                                           name=f"l0_{q}_{j}")
                        nc.vector.tensor_add(pair[:], efs[q * JB + j - 1][:],
                                             ef[:])
                        tree_push(q, 1, pair)
                elif j == ZRUN_J:
                    # merge pending counter partials (sum of 0..23), then run
                    zq = st["zq"] = ztpool.tile([128, QW], BF16, tag="zq",
                                                name=f"zq_{q}")
                    p4, p3 = st["pend"].pop(4), st["pend"].pop(3)
                    nc.vector.tensor_add(zq[:], p4[:], p3[:])
                    nc.vector.tensor_add(zq[:], zq[:], ef[:])
                elif q < NQ - 1 or j < JB - 2:
                    if q == NQ - 1 and j >= JB - 4:
                        # halved so zq subtile closes track the exps
                        for s in range(2):
                            sl = slice(s * 512, (s + 1) * 512)
                            nc.vector.tensor_add(st["zq"][:, sl],
                                                 st["zq"][:, sl], ef[:, sl])
                    else:
                        nc.vector.tensor_add(st["zq"][:], st["zq"][:], ef[:])
                elif j == JB - 2:
                    for s in range(2):
                        sl = slice(s * 512, (s + 1) * 512)
                        nc.vector.tensor_add(st["zq"][:, sl],
                                             st["zq"][:, sl], ef[:, sl])
                else:
                    # last tile of the run: zq(0..30) closes on PE (ones-MM
                    # partial) and ef31 accumulates straight into pzt, so
                    # no DVE Z work trails the last exp and the tail needs
                    # no zclose.
                    for s in range(2):
                        pzt = pw.tile([1, 512], F32, tag="pw",
                                      name=f"pz3_{s}")
                        st["pzt"][s] = pzt
                        nc.tensor.matmul(
                            pzt[:], ones_bf[:],
                            st["zq"][:, s * 512:(s + 1) * 512],
                            start=True, stop=False)
                    for s in range(2):
                        nc.tensor.matmul(
                            st["pzt"][s][:], ones_bf[:],
                            ef[:, s * 512:(s + 1) * 512],
                            start=False, stop=True)

            def emit_dve_tile(t, ef):
                # DVE fast-exp tiles get their own pw-pool PSUM halves so
                # the slow (1.2us) DVE read never stalls the pf rotation
                # that feeds the Scalar exp pipe (measured ~2us bubbles).
                q, j = divmod(t, JB)
                i0 = q * QW
                for s in range(2):
                    pp = pw.tile([128, 512], F32, tag="pw",
                                 name=f"fd_{t}_{s}")
                    nc.tensor.matmul(
                        pp[:], ph_sb[:, j * 128:(j + 1) * 128],
                        th_sb[:, i0 + s * 512:i0 + (s + 1) * 512],
                        start=True, stop=True)
                    nc.vector.tensor_scalar(
                        ef[:, s * 512:(s + 1) * 512].bitcast(I16), pp[:],
                        float(CB), 0.0, OP.add, OP.max)

            def emit_f(t, pft, s):
                q, j = divmod(t, JB)
                i0 = q * QW
                nc.tensor.matmul(
                    pft[:, s * 512:(s + 1) * 512],
                    ph_sb[:, j * 128:(j + 1) * 128],
                    th_sb[:, i0 + s * 512:i0 + (s + 1) * 512],
                    start=True, stop=True)

            def emit_y(ty):
                qy, jy = divmod(ty, JB)
                if jy == 0:
                    state[qy]["pyt"] = py.tile([128, QW], F32, tag="py",
                                               name=f"py_{qy}")
                efy = efs.pop(ty)
                for s in range(2):
                    nc.tensor.matmul(
                        state[qy]["pyt"][:, s * 512:(s + 1) * 512],
                        gT_sb[:, jy * 128:(jy + 1) * 128],
                        efy[:, s * 512:(s + 1) * 512],
                        start=(jy == 0), stop=(jy == JB - 1))

            # ---------------- pre-loop: th0 / ph0 / th1, split t=0,1 ------
            # PE-queue order matters (in-order engine): f0s0 is emitted as
            # soon as its gates (th0a/th0b/ph0a) are, so exp0a isn't stuck
            # behind ph0b/th1 matmuls.
            state[0] = {"pzt": [None, None], "zbt": [None, None], "pend": {}}
            pf0 = pf.tile([128, QW], F32, tag="pf", name="pf_0")
            pf1 = pf.tile([128, QW], F32, tag="pf", name="pf_1")
            ef0 = efpool.tile([128, QW], BF16, tag="ef", name="ef_0")
            ef1 = efpool.tile([128, QW], BF16, tag="ef", name="ef_1")
            efs[0], efs[1] = ef0, ef1

            th_piece(0, 256, ev="s", pool_tag="pw")
            th_piece(256, 256, ev="v", pool_tag="pw")
            ph_piece(0, 128, ev="v", pool_tag="pw")
            emit_f(0, pf0, 0)
            nc.scalar.activation(ef0[:, 0:512], pf0[:, 0:512], AF.Exp,
                                 bias=aux[:, 5:6], scale=float(SCL))
            ph_piece(128, 384, ev="v", pool_tag="pw")
            emit_f(1, pf1, 0)
            nc.scalar.activation(ef1[:, 0:512], pf1[:, 0:512], AF.Exp,
                                 bias=aux[:, 5:6], scale=float(SCL))
            # s1 halves gated on th chunk 1 (x cols 512:1024)
            th_piece(512, 512, ev="v", pool_tag="pw")
            emit_f(0, pf0, 1)
            emit_f(1, pf1, 1)
            nc.scalar.activation(ef0[:, 512:1024], pf0[:, 512:1024], AF.Exp,
                                 bias=aux[:, 5:6], scale=float(SCL))
            nc.scalar.activation(ef1[:, 512:1024], pf1[:, 512:1024], AF.Exp,
                                 bias=aux[:, 5:6], scale=float(SCL))

            # deferred DMA issues: each issue instruction occupies the
            # queue engine (and blocks on queue-slot availability), so
            # only the pre-loop-critical transfers go ahead of the
            # evictions/exps above.
            nc.sync.dma_start(x0[:, 1024:2048], x_d[0:128, 1024:2048])
            nc.sync.dma_start(x1[:, 1024:2048], x_d[128:256, 1024:2048])
            nc.sync.dma_start(gw[:, 0:CI], gw_d[0:128, :])
            nc.sync.dma_start(gw[:, CI:2 * CI], gw_d[128:256, :])
            nc.sync.dma_start(ww[:], ww_d[:])
            nc.sync.dma_start(x0[:, 2048:4096], x_d[0:128, 2048:4096])
            nc.sync.dma_start(x1[:, 2048:4096], x_d[128:256, 2048:4096])

            # sched ops of t<2 (the main loop starts at t=2): ph chunk 1
            for op in sched.get(0, []) + sched.get(1, []):
                emit_sched_op(op)

            # ---------------- main flat pipeline ----------------
            y_ptr = 0
            for t in range(2, T):
                q, j = divmod(t, JB)
                if j == 0:
                    state[q] = {"pzt": [None, None], "zbt": [None, None],
                                "pend": {}}
                st = state[q]
                ef = efpool.tile([128, QW], BF16, tag="ef", name=f"ef_{t}")
                efs[t] = ef
                if (q, j) in DVE_SET:
                    emit_dve_tile(t, ef)
                else:
                    pft = pf.tile([128, QW], F32, tag="pf", name=f"pf_{t}")
                    emit_f(t, pft, 0)
                    emit_f(t, pft, 1)
                    nc.scalar.activation(ef[:], pft[:], AF.Exp,
                                         bias=aux[:, 5:6], scale=float(SCL))
                if t == 3:
                    z_accum(0, 1, efs[1])   # deferred j=1 pair (split exps)
                z_accum(q, j, ef)
                # previous quarter's deferred work
                if q > 0:
                    if j == ZCLOSE_J[0]:
                        zclose(q - 1, 0)
                    elif j == ZCLOSE_J[1]:
                        zclose(q - 1, 1)
                    elif j == ZINV_J:
                        zinv(q - 1, 0)
                        zinv(q - 1, 1)
                    elif j == BCAST_J[0]:
                        bcast(q - 1, 0)
                    elif j == BCAST_J[1]:
                        bcast(q - 1, 1)
                    elif j == MULT_J[0]:
                        mult(q - 1, 0)
                    elif j == MULT_J[1]:
                        mult(q - 1, 1)
                    elif WPROJ_J0 <= j < WPROJ_J0 + 4:
                        wproj(q - 1, (0, 2, 1, 3)[j - WPROJ_J0])
                for op in sched.get(t, []):
                    emit_sched_op(op)
                # trailing y accumulation (with q3 lag rampdown)
                def y_ok():
                    return not (y_ptr % JB in (0, 1) and j < 13)
                if t - YLAG >= 0 and y_ptr <= t - YLAG and y_ok():
                    emit_y(y_ptr)
                    y_ptr += 1
                    # catch up after a quarter-boundary hold
                    if y_ptr <= t - YLAG and y_ok():
                        emit_y(y_ptr)
                        y_ptr += 1
                if q == NQ - 1 and j in YEXTRA and y_ptr <= t - 2 and y_ok():
                    emit_y(y_ptr)
                    y_ptr += 1

            # ---------------- last quarter's tail (pipelined by half) ------
            q = NQ - 1
            pending = list(range(y_ptr, T))

            def y_half(ty, s):
                qy, jy = divmod(ty, JB)
                nc.tensor.matmul(
                    state[qy]["pyt"][:, s * 512:(s + 1) * 512],
                    gT_sb[:, jy * 128:(jy + 1) * 128],
                    efs[ty][:, s * 512:(s + 1) * 512],
                    start=(jy == 0), stop=(jy == JB - 1))

            # s0 y-drain, then the Z chains for both halves as early as
            # their deps allow (zq-s halves close right after the last
            # exp), with the s1 y-drain interleaved behind zclose(s1).
            for ty in pending:
                y_half(ty, 0)
            zinv(q, 0)
            zinv(q, 1)
            bcast(q, 0)
            for ty in pending:
                y_half(ty, 1)
            bcast(q, 1)
            mult(q, 0)
            wproj(q, 0, nc.scalar)
            wproj(q, 2)
            mult(q, 1)
            wproj(q, 1, nc.scalar)
            wproj(q, 3)

    nc.compile()
    return nc


_CACHE = {}


def _get_nc():
    if "nc" not in _CACHE:
        _CACHE["nc"] = build()
    return _CACHE["nc"]


def _in_maps(x, g_w, g_b, theta_w, theta_b, phi_w, phi_b, W_w, W_b):
    x = np.ascontiguousarray(np.asarray(x, dtype=np.float32))
    wbe = (np.asarray(W_w, np.float32) @ np.asarray(g_b, np.float32)
           + np.asarray(W_b, np.float32))
    common = {
        "thw_t": np.ascontiguousarray(np.asarray(theta_w, np.float32).T),
        "phw_t": np.ascontiguousarray(np.asarray(phi_w, np.float32).T),
        "gw_t": np.ascontiguousarray(np.asarray(g_w, np.float32).T),
        "ww_t": np.ascontiguousarray(np.asarray(W_w, np.float32).T),
        "aux": np.stack(
            [
                np.asarray(theta_b, np.float32) * np.float32(A_SCALE),
                np.asarray(phi_b, np.float32),
                wbe[:128],
                wbe[128:],
                np.ones(128, np.float32),
                np.full(128, EXP_BIAS, np.float32),
            ],
            axis=1,
        ),
    }
    return [
        {"x": np.ascontiguousarray(x[b].reshape(C, N)), **common}
        for b in range(B)
    ]


def run(in_maps, **kw):
    nc = _get_nc()
    return run_bass_kernel_spmd(nc, in_maps, list(range(B)), **kw)


def kernel(**inputs):
    res = run(_in_maps(**inputs))
    out = np.stack([res.results[b]["out"] for b in range(B)])
    return out.reshape(B, C, H, Wd)
